# revision 4
# baseline (speedup 1.0000x reference)
"""GATv2 x3 + GraphNorm + mean-pool + linear on 8 Trainium2 cores.

Structure exploited: 4096 disjoint fully-connected 22-node graphs.
Sharding: 512 graphs per core (data parallel); weights replicated.
Outputs are written feature-major (alphaT [4,E], pooledT, oT) and
transposed/reordered on the host during unsharding.
"""
import numpy as np

N_CORES = 8
NPG = 22
E1 = NPG * NPG            # 484
H, C, F1 = 4, 16, 64
NG = 4096 // N_CORES      # 512 graphs/core
NNODE = NG * NPG          # 11264 node cols/core
GBLK = 16                 # graphs per block
NBLK = NG // GBLK         # 32
WBLK = GBLK * NPG         # 352 node cols/block
EBLK = GBLK * E1          # 7744 pair cols/block
EC = NG * E1              # 247808 edges/core
EPS_GN = 1e-5

_PROG = {}


def _build_program():
    import concourse.bass as bass
    import concourse.tile as tile
    import concourse.mybir as mybir
    from concourse import bacc

    F32 = mybir.dt.float32
    BF16 = mybir.dt.bfloat16
    AF = mybir.ActivationFunctionType
    OP = mybir.AluOpType
    AX = mybir.AxisListType

    nc = bacc.Bacc(None, target_bir_lowering=False)

    x_d = nc.dram_tensor("x", [NNODE, 22], F32, kind="ExternalInput")
    wl_d, wr_d, sgn_d, aa_d, bias_d, gnw_d, gnb_d, gnm_d = [], [], [], [], [], [], [], []
    for l in range(3):
        D = 22 if l == 0 else 64
        wl_d.append(nc.dram_tensor(f"wl{l}", [D + 1, 68], F32, kind="ExternalInput"))
        wr_d.append(nc.dram_tensor(f"wr{l}", [D + 1, 64], F32, kind="ExternalInput"))
        sgn_d.append(nc.dram_tensor(f"sgn{l}", [64, 64], BF16, kind="ExternalInput"))
        aa_d.append(nc.dram_tensor(f"aa{l}", [64, 1], F32, kind="ExternalInput"))
        bias_d.append(nc.dram_tensor(f"bias{l}", [128, 1], F32, kind="ExternalInput"))
        gnw_d.append(nc.dram_tensor(f"gnw{l}", [128, 1], F32, kind="ExternalInput"))
        gnb_d.append(nc.dram_tensor(f"gnb{l}", [128, 1], F32, kind="ExternalInput"))
        gnm_d.append(nc.dram_tensor(f"gnm{l}", [128, 1], F32, kind="ExternalInput"))
    linw_d = nc.dram_tensor("linw", [128, 2], F32, kind="ExternalInput")
    linb_d = nc.dram_tensor("linb", [2, 1], F32, kind="ExternalInput")
    ident_d = nc.dram_tensor("ident", [128, 128], F32, kind="ExternalInput")

    alphat_d = nc.dram_tensor("alphat", [4, EC], F32, kind="ExternalOutput")
    pooledt_d = nc.dram_tensor("pooledt", [128, 256], F32, kind="ExternalOutput")
    ot_d = nc.dram_tensor("ot", [2, 512], F32, kind="ExternalOutput")

    with tile.TileContext(nc) as tc:
        dram = tc.alloc_tile_pool(name="dram", bufs=1, space="DRAM")
        xt_a = dram.tile([65, NNODE], F32)
        xt_b = dram.tile([65, NNODE], F32)
        eal_scr = dram.tile([64, NNODE + NPG], F32)
        EALW = NNODE + NPG

        consts = tc.alloc_tile_pool(name="consts", bufs=1)
        wl_t, wr_t, sgn_t, aa_t, bias_t, gnw_t, gnb_t, gnm_t = [], [], [], [], [], [], [], []
        for l in range(3):
            D = 22 if l == 0 else 64
            t = consts.tile([D + 1, 68], F32, tag=f"wl{l}")
            nc.sync.dma_start(out=t[:], in_=wl_d[l][:]); wl_t.append(t)
            t = consts.tile([D + 1, 64], F32, tag=f"wr{l}")
            nc.sync.dma_start(out=t[:], in_=wr_d[l][:]); wr_t.append(t)
            t = consts.tile([64, 64], BF16, tag=f"sgn{l}")
            nc.sync.dma_start(out=t[:], in_=sgn_d[l][:]); sgn_t.append(t)
            for lst, dd, tg in ((aa_t, aa_d, "aa"), (bias_t, bias_d, "bias"),
                                (gnw_t, gnw_d, "gnw"), (gnb_t, gnb_d, "gnb"),
                                (gnm_t, gnm_d, "gnm")):
                shp = [64, 1] if tg == "aa" else [128, 1]
                t = consts.tile(shp, F32, tag=f"{tg}{l}")
                nc.sync.dma_start(out=t[:], in_=dd[l][:]); lst.append(t)
        linw_t = consts.tile([128, 2], F32)
        nc.sync.dma_start(out=linw_t[:], in_=linw_d[:])
        linb_t = consts.tile([2, 1], F32)
        nc.sync.dma_start(out=linb_t[:], in_=linb_d[:])
        ident_t = consts.tile([128, 128], F32)
        nc.sync.dma_start(out=ident_t[:], in_=ident_d[:])

        # ---- Stage 0: XT1 = x^T into xt_a rows 0:22; ones rows ----
        with tc.tile_pool(name="tr_sb", bufs=3) as trp, \
             tc.tile_pool(name="tr_ps", bufs=3, space="PSUM") as trps:
            onesrow = trp.tile([1, 1408], F32)
            nc.vector.memset(onesrow[:], 1.0)
            for t8 in range(8):
                sl = slice(t8 * 1408, (t8 + 1) * 1408)
                nc.sync.dma_start(out=xt_a[22:23, sl], in_=onesrow[:])
                nc.sync.dma_start(out=xt_a[64:65, sl], in_=onesrow[:])
                nc.sync.dma_start(out=xt_b[64:65, sl], in_=onesrow[:])
            for t in range(NNODE // 128):
                xm = trp.tile([128, 22], F32, tag="xm")
                nc.sync.dma_start(out=xm[:], in_=x_d[t * 128:(t + 1) * 128, :])
                pt = trps.tile([22, 128], F32, tag="pt")
                nc.tensor.transpose(pt[:], xm[:], ident_t[:])
                st = trp.tile([22, 128], F32, tag="st")
                nc.vector.tensor_copy(out=st[:], in_=pt[:])
                nc.sync.dma_start(out=xt_a[0:22, t * 128:(t + 1) * 128], in_=st[:])

        # ---- Layers ----
        for l in range(3):
            D = 22 if l == 0 else 64
            DK = D + 1
            src = xt_a if l % 2 == 0 else xt_b
            dst = xt_b if l % 2 == 0 else xt_a
            last = l == 2

            lp = tc.alloc_tile_pool(name=f"lay{l}", bufs=1)
            w_stack = lp.tile([128, NBLK * 176], F32)   # [(g2,hc), (blk,kk,j)]
            blkp = tc.alloc_tile_pool(name=f"blk{l}", bufs=2)
            zp = tc.alloc_tile_pool(name=f"z{l}", bufs=1)
            psA = tc.alloc_tile_pool(name=f"psA{l}", bufs=2, space="PSUM")
            psB = tc.alloc_tile_pool(name=f"psB{l}", bufs=1, space="PSUM")

            for blk in range(NBLK):
                c0 = blk * WBLK
                # --- projections ---
                xt_c = blkp.tile([DK, WBLK], F32, tag="xt")
                nc.sync.dma_start(out=xt_c[:], in_=src[0:DK, c0:c0 + WBLK])
                mmL = psA.tile([68, WBLK], F32, tag="mmL")
                nc.tensor.matmul(mmL[:], wl_t[l][:], xt_c[:], start=True, stop=True)
                mmR = psA.tile([64, WBLK], F32, tag="mmR")
                nc.tensor.matmul(mmR[:], wr_t[l][:], xt_c[:], start=True, stop=True)
                xla = blkp.tile([64, WBLK], BF16, tag="xla")
                nc.vector.tensor_scalar_mul(xla[:], mmL[0:64, :], aa_t[l][:])
                xra = blkp.tile([64, WBLK], BF16, tag="xra")
                nc.vector.tensor_scalar_mul(xra[:], mmR[:], aa_t[l][:])
                xlt = blkp.tile([128, WBLK], F32, tag="xlt")
                nc.vector.tensor_copy(out=xlt[0:64, :], in_=mmL[0:64, :])
                nc.sync.dma_start(out=xlt[64:128, 0:330], in_=xlt[0:64, 22:352])
                # exp(0.6*al): expand (h)->(h,c) rows via DRAM, reload shifted
                eal4 = blkp.tile([68, WBLK], F32, tag="eal4")
                nc.scalar.activation(out=eal4[64:68, :], in_=mmL[64:68, :],
                                     func=AF.Exp, scale=0.6)
                nc.sync.dma_start(
                    out=eal_scr[:, c0:c0 + WBLK].rearrange("(h c) w -> h c w", c=16),
                    in_=eal4[64:68, :].unsqueeze(1).broadcast_to([4, 16, WBLK]))
                ealrep = blkp.tile([128, WBLK], F32, tag="ealrep")
                in_rep = bass.AP(eal_scr.tensor, eal_scr.offset + c0,
                                 [[NPG, 2], [EALW, 64], [1, WBLK]])
                nc.sync.dma_start(out=ealrep[:], in_=in_rep)

                # --- pairwise scores ---
                p2 = blkp.tile([64, EBLK], BF16, tag="p2")
                in0 = xla[:].rearrange("p (g i) -> p g i", i=NPG) \
                    .unsqueeze(3).broadcast_to([64, GBLK, NPG, NPG])
                in1 = xra[:].rearrange("p (g j) -> p g j", j=NPG) \
                    .unsqueeze(2).broadcast_to([64, GBLK, NPG, NPG])
                nc.vector.tensor_tensor(out=p2[:], in0=in0, in1=in1, op=OP.add)
                nc.scalar.activation(out=p2[:], in_=p2[:], func=AF.Abs)

                ea = blkp.tile([128, 3872], F32, tag="ea")
                for grp in range(2):
                    sps = psB.tile([128, 4, 512], F32, tag="sps")
                    for gg in range(8):
                        g = grp * 8 + gg
                        out_ap = sps[(g % 2) * 64:(g % 2) * 64 + 64, gg // 2, 0:E1]
                        nc.tensor.matmul(out_ap, sgn_t[l][:],
                                         p2[:, g * E1:(g + 1) * E1],
                                         start=True, stop=True)
                    nc.scalar.activation(
                        out=ea[:, grp * 1936:(grp + 1) * 1936]
                            .rearrange("p (k e) -> p k e", e=E1),
                        in_=sps[:, :, 0:E1], func=AF.Exp, scale=0.4)
                # ea cols: (kk: 8, e1: 484), graph = blk*16 + 2*kk + g2(row half)
                ealv = ealrep[:].rearrange("p (k s) -> p k s", s=44)[:, :, 0:NPG] \
                    .unsqueeze(3).broadcast_to([128, 8, NPG, NPG])
                nc.vector.tensor_tensor(out=ea[:], in0=ea[:], in1=ealv, op=OP.mult)

                den = blkp.tile([128, 176], F32, tag="den")
                nc.vector.tensor_reduce(
                    out=den[:],
                    in_=ea[:].rearrange("p (k i j) -> p k j i", i=NPG, j=NPG),
                    axis=AX.X, op=OP.add)
                rden = blkp.tile([128, 176], F32, tag="rden")
                nc.vector.reciprocal(out=rden[:], in_=den[:])

                if last:
                    alp = zp.tile([128, 3872], F32, tag="alp")
                    rdv = rden[:].rearrange("p (k j) -> p k j", j=NPG) \
                        .unsqueeze(2).broadcast_to([128, 8, NPG, NPG])
                    nc.vector.tensor_tensor(out=alp[:], in0=ea[:], in1=rdv, op=OP.mult)
                    for g2 in range(2):
                        for h in range(4):
                            r = g2 * 64 + h * 16
                            in_a = alp[r:r + 1, :].rearrange("p (k e) -> p k e", e=E1)
                            out_a = bass.AP(alphat_d,
                                            h * EC + (blk * 16 + g2) * E1,
                                            [[2 * E1, 8], [1, E1]])
                            nc.sync.dma_start(out=out_a, in_=in_a)

                z = zp.tile([128, 3872], F32, tag="z")
                xltv = xlt[:].rearrange("p (k s) -> p k s", s=44)[:, :, 0:NPG] \
                    .unsqueeze(3).broadcast_to([128, 8, NPG, NPG])
                nc.vector.tensor_tensor(out=z[:], in0=xltv, in1=ea[:], op=OP.mult)
                wsl = w_stack[:, blk * 176:(blk + 1) * 176]
                nc.vector.tensor_reduce(
                    out=wsl,
                    in_=z[:].rearrange("p (k i j) -> p k j i", i=NPG, j=NPG),
                    axis=AX.X, op=OP.add)
                nc.vector.tensor_tensor(out=wsl, in0=wsl, in1=rden[:], op=OP.mult)
                if last:
                    nc.vector.tensor_scalar_add(wsl, wsl, bias_t[l][:])
                else:
                    nc.vector.tensor_scalar(out=wsl, in0=wsl, scalar1=bias_t[l][:],
                                            scalar2=0.0, op0=OP.add, op1=OP.max)

            psB.release()
            psA.release()
            zp.release()
            blkp.release()

            # --- GraphNorm over w_stack [128, (blk,kk,j)] ---
            gnp = tc.alloc_tile_pool(name=f"gn{l}", bufs=1)
            NCG = NBLK * 8  # 256 graph-cols (pairs of graphs per row-half)
            wv = w_stack[:].rearrange("p (G j) -> p G j", j=NPG)
            mean = gnp.tile([128, NCG], F32)
            nc.vector.tensor_reduce(out=mean[:], in_=wv, axis=AX.X, op=OP.add)
            mmean = gnp.tile([128, NCG], F32)
            nc.vector.tensor_scalar_mul(mmean[:], mean[:], gnm_t[l][:])
            nc.vector.tensor_scalar_mul(mmean[:], mmean[:], 1.0 / NPG)
            cent = gnp.tile([128, NBLK * 176], F32)
            nc.vector.tensor_tensor(
                out=cent[:].rearrange("p (G j) -> p G j", j=NPG), in0=wv,
                in1=mmean[:].unsqueeze(2).broadcast_to([128, NCG, NPG]),
                op=OP.subtract)
            sq = gnp.tile([128, NBLK * 176], F32)
            nc.vector.tensor_tensor(out=sq[:], in0=cent[:], in1=cent[:], op=OP.mult)
            var = gnp.tile([128, NCG], F32)
            nc.vector.tensor_reduce(
                out=var[:], in_=sq[:].rearrange("p (G j) -> p G j", j=NPG),
                axis=AX.X, op=OP.add)
            nc.vector.tensor_scalar(out=var[:], in0=var[:], scalar1=1.0 / NPG,
                                    scalar2=EPS_GN, op0=OP.mult, op1=OP.add)
            sd = gnp.tile([128, NCG], F32)
            nc.scalar.activation(out=sd[:], in_=var[:], func=AF.Sqrt)
            inv = gnp.tile([128, NCG], F32)
            nc.vector.reciprocal(out=inv[:], in_=sd[:])
            winv = gnp.tile([128, NCG], F32)
            nc.vector.tensor_scalar_mul(winv[:], inv[:], gnw_t[l][:])
            gout = gnp.tile([128, NBLK * 176], F32)
            nc.vector.tensor_tensor(
                out=gout[:].rearrange("p (G j) -> p G j", j=NPG),
                in0=cent[:].rearrange("p (G j) -> p G j", j=NPG),
                in1=winv[:].unsqueeze(2).broadcast_to([128, NCG, NPG]), op=OP.mult)
            nc.vector.tensor_scalar_add(gout[:], gout[:], gnb_t[l][:])

            if not last:
                # XT-next: node col = (2*G + g2)*22 + j = G*44 + g2*22 + j
                for g2 in range(2):
                    in_g = gout[g2 * 64:g2 * 64 + 64, :] \
                        .rearrange("p (G j) -> p G j", j=NPG)
                    out_g = bass.AP(dst.tensor, dst.offset + g2 * NPG,
                                    [[NNODE, 64], [44, NCG], [1, NPG]])
                    nc.sync.dma_start(out=out_g, in_=in_g)
            else:
                pooled_t = gnp.tile([128, NCG], F32)
                nc.vector.tensor_reduce(
                    out=pooled_t[:], in_=gout[:].rearrange("p (G j) -> p G j", j=NPG),
                    axis=AX.X, op=OP.add)
                nc.vector.tensor_scalar_mul(pooled_t[:], pooled_t[:], 1.0 / NPG)
                nc.sync.dma_start(out=pooledt_d[:], in_=pooled_t[:])
                with tc.tile_pool(name="ops", bufs=1, space="PSUM") as opsps:
                    osb = gnp.tile([2, 512], F32)
                    for g2 in range(2):
                        ops = opsps.tile([2, NCG], F32, tag=f"o{g2}")
                        nc.tensor.matmul(ops[:], linw_t[g2 * 64:g2 * 64 + 64, :],
                                         pooled_t[g2 * 64:g2 * 64 + 64, :],
                                         start=True, stop=True)
                        nc.vector.tensor_scalar_add(
                            osb[:, g2 * NCG:(g2 + 1) * NCG], ops[:], linb_t[:])
                    nc.sync.dma_start(out=ot_d[:], in_=osb[:])
            gnp.release()
            lp.release()

        consts.release()
        dram.release()

    nc.finalize()
    return nc


def _preprocess(inputs):
    import ml_dtypes
    x = np.ascontiguousarray(np.asarray(inputs["x"], np.float32))
    common = {}
    for li, l in enumerate(("1", "2", "3")):
        Wl = np.asarray(inputs[f"Wl{l}"], np.float32)
        bl = np.asarray(inputs[f"bl{l}"], np.float32)
        Wr = np.asarray(inputs[f"Wr{l}"], np.float32)
        br = np.asarray(inputs[f"br{l}"], np.float32)
        att = np.asarray(inputs[f"att{l}"], np.float32)   # [H, C]
        WlA = np.einsum("dhc,hc->dh", Wl.reshape(-1, H, C), att)
        blA = np.einsum("hc,hc->h", bl.reshape(H, C), att)
        common[f"wl{li}"] = np.concatenate(
            [np.vstack([Wl, bl[None, :]]),
             np.vstack([WlA, blA[None, :]])], axis=1).astype(np.float32)
        common[f"wr{li}"] = np.vstack([Wr, br[None, :]]).astype(np.float32)
        sgn = np.zeros((64, 64), np.float32)
        s = np.sign(att)
        for h in range(H):
            sgn[h * C:(h + 1) * C, h * C:(h + 1) * C] = s[h][:, None]
        common[f"sgn{li}"] = sgn.astype(ml_dtypes.bfloat16)
        common[f"aa{li}"] = np.abs(att).reshape(64, 1).astype(np.float32)
        common[f"bias{li}"] = np.tile(np.asarray(inputs[f"bias{l}"], np.float32), 2)[:, None].copy()
        common[f"gnw{li}"] = np.tile(np.asarray(inputs[f"gnw{l}"], np.float32), 2)[:, None].copy()
        common[f"gnb{li}"] = np.tile(np.asarray(inputs[f"gnb{l}"], np.float32), 2)[:, None].copy()
        common[f"gnm{li}"] = np.tile(np.asarray(inputs[f"gnm{l}"], np.float32), 2)[:, None].copy()
    common["linw"] = np.tile(np.asarray(inputs["linW"], np.float32), (2, 1)).copy()
    common["linb"] = np.asarray(inputs["linb"], np.float32).reshape(2, 1).copy()
    common["ident"] = np.eye(128, dtype=np.float32)
    in_maps = []
    for c in range(N_CORES):
        m = dict(common)
        m["x"] = x[c * NNODE:(c + 1) * NNODE].copy()
        in_maps.append(m)
    return in_maps


def kernel(**inputs):
    from concourse.bass_utils import run_bass_kernel_spmd

    if "nc" not in _PROG:
        _PROG["nc"] = _build_program()
    nc = _PROG["nc"]
    in_maps = _preprocess(inputs)
    res = run_bass_kernel_spmd(nc, in_maps, list(range(N_CORES))).results

    o = np.empty((4096, 2), np.float32)
    pooled = np.empty((4096, F1), np.float32)
    alpha3 = np.empty((4096 * E1, H), np.float32)
    bg = np.arange(256)
    for c in range(N_CORES):
        r = res[c]
        alpha3[c * EC:(c + 1) * EC] = r["alphat"].T
        pt = r["pooledt"]
        ot = r["ot"]
        for g2 in range(2):
            g = c * NG + 2 * bg + g2
            pooled[g] = pt[g2 * 64:(g2 + 1) * 64, :][:, bg].T
            o[g] = ot[:, g2 * 256 + bg].T
    return (o, pooled, alpha3)


# revision 6
# speedup vs baseline: 1.1901x; 1.1901x over previous
"""GATv2 x3 + GraphNorm + mean-pool + linear on 8 Trainium2 cores.

Structure exploited: 4096 disjoint fully-connected 22-node graphs.
Sharding: 512 graphs per core (data parallel); weights replicated.
Outputs are written feature-major (alphaT [4,E], pooledT, oT) and
transposed/reordered on the host during unsharding.
"""
import numpy as np

N_CORES = 8
NPG = 22
E1 = NPG * NPG            # 484
H, C, F1 = 4, 16, 64
NG = 4096 // N_CORES      # 512 graphs/core
NNODE = NG * NPG          # 11264 node cols/core
GBLK = 16                 # graphs per block
NBLK = NG // GBLK         # 32
WBLK = GBLK * NPG         # 352 node cols/block
EBLK = GBLK * E1          # 7744 pair cols/block
EC = NG * E1              # 247808 edges/core
EPS_GN = 1e-5

_PROG = {}


def _build_program():
    import concourse.bass as bass
    import concourse.tile as tile
    import concourse.mybir as mybir
    from concourse import bacc

    F32 = mybir.dt.float32
    BF16 = mybir.dt.bfloat16
    AF = mybir.ActivationFunctionType
    OP = mybir.AluOpType
    AX = mybir.AxisListType

    nc = bacc.Bacc(None, target_bir_lowering=False)

    x_d = nc.dram_tensor("x", [NNODE, 22], F32, kind="ExternalInput")
    wl_d, wr_d, sgn_d, aa_d, bias_d, gnw_d, gnb_d, gnm_d = [], [], [], [], [], [], [], []
    alw_d = []
    for l in range(3):
        D = 22 if l == 0 else 64
        wl_d.append(nc.dram_tensor(f"wl{l}", [D + 1, 68], F32, kind="ExternalInput"))
        wr_d.append(nc.dram_tensor(f"wr{l}", [D + 1, 64], F32, kind="ExternalInput"))
        sgn_d.append(nc.dram_tensor(f"sgn{l}", [128, 64], BF16, kind="ExternalInput"))
        alw_d.append(nc.dram_tensor(f"alw{l}", [68, 64], BF16, kind="ExternalInput"))
        aa_d.append(nc.dram_tensor(f"aa{l}", [64, 1], F32, kind="ExternalInput"))
        bias_d.append(nc.dram_tensor(f"bias{l}", [128, 1], F32, kind="ExternalInput"))
        gnw_d.append(nc.dram_tensor(f"gnw{l}", [128, 1], F32, kind="ExternalInput"))
        gnb_d.append(nc.dram_tensor(f"gnb{l}", [128, 1], F32, kind="ExternalInput"))
        gnm_d.append(nc.dram_tensor(f"gnm{l}", [128, 1], F32, kind="ExternalInput"))
    linw_d = nc.dram_tensor("linw", [128, 2], F32, kind="ExternalInput")
    linb_d = nc.dram_tensor("linb", [2, 1], F32, kind="ExternalInput")
    ident_d = nc.dram_tensor("ident", [128, 128], F32, kind="ExternalInput")

    alphat_d = nc.dram_tensor("alphat", [4, EC], F32, kind="ExternalOutput")
    pooledt_d = nc.dram_tensor("pooledt", [128, 256], F32, kind="ExternalOutput")
    ot_d = nc.dram_tensor("ot", [2, 512], F32, kind="ExternalOutput")

    with tile.TileContext(nc) as tc:
        dram = tc.alloc_tile_pool(name="dram", bufs=1, space="DRAM")
        xt_a = dram.tile([65, NNODE], F32)
        xt_b = dram.tile([65, NNODE], F32)

        consts = tc.alloc_tile_pool(name="consts", bufs=1)
        wl_t, wr_t, sgn_t, aa_t, bias_t, gnw_t, gnb_t, gnm_t = [], [], [], [], [], [], [], []
        alw_t = []
        for l in range(3):
            D = 22 if l == 0 else 64
            t = consts.tile([D + 1, 68], F32, tag=f"wl{l}")
            nc.sync.dma_start(out=t[:], in_=wl_d[l][:]); wl_t.append(t)
            t = consts.tile([D + 1, 64], F32, tag=f"wr{l}")
            nc.sync.dma_start(out=t[:], in_=wr_d[l][:]); wr_t.append(t)
            t = consts.tile([128, 64], BF16, tag=f"sgn{l}")
            nc.sync.dma_start(out=t[:], in_=sgn_d[l][:]); sgn_t.append(t)
            t = consts.tile([68, 64], BF16, tag=f"alw{l}")
            nc.sync.dma_start(out=t[:], in_=alw_d[l][:]); alw_t.append(t)
            for lst, dd, tg in ((aa_t, aa_d, "aa"), (bias_t, bias_d, "bias"),
                                (gnw_t, gnw_d, "gnw"), (gnb_t, gnb_d, "gnb"),
                                (gnm_t, gnm_d, "gnm")):
                shp = [64, 1] if tg == "aa" else [128, 1]
                t = consts.tile(shp, F32, tag=f"{tg}{l}")
                nc.sync.dma_start(out=t[:], in_=dd[l][:]); lst.append(t)
        linw_t = consts.tile([128, 2], F32)
        nc.sync.dma_start(out=linw_t[:], in_=linw_d[:])
        linb_t = consts.tile([2, 1], F32)
        nc.sync.dma_start(out=linb_t[:], in_=linb_d[:])
        ident_t = consts.tile([128, 128], F32)
        nc.sync.dma_start(out=ident_t[:], in_=ident_d[:])

        # ---- Stage 0: XT1 = x^T into xt_a rows 0:22; ones rows ----
        with tc.tile_pool(name="tr_sb", bufs=3) as trp, \
             tc.tile_pool(name="tr_ps", bufs=3, space="PSUM") as trps:
            onesrow = trp.tile([1, 1408], F32)
            nc.vector.memset(onesrow[:], 1.0)
            for t8 in range(8):
                sl = slice(t8 * 1408, (t8 + 1) * 1408)
                nc.sync.dma_start(out=xt_a[22:23, sl], in_=onesrow[:])
                nc.sync.dma_start(out=xt_a[64:65, sl], in_=onesrow[:])
                nc.sync.dma_start(out=xt_b[64:65, sl], in_=onesrow[:])
            for t in range(NNODE // 128):
                xm = trp.tile([128, 22], F32, tag="xm")
                nc.sync.dma_start(out=xm[:], in_=x_d[t * 128:(t + 1) * 128, :])
                pt = trps.tile([22, 128], F32, tag="pt")
                nc.tensor.transpose(pt[:], xm[:], ident_t[:])
                st = trp.tile([22, 128], F32, tag="st")
                nc.vector.tensor_copy(out=st[:], in_=pt[:])
                nc.sync.dma_start(out=xt_a[0:22, t * 128:(t + 1) * 128], in_=st[:])

        # ---- Layers ----
        for l in range(3):
            D = 22 if l == 0 else 64
            DK = D + 1
            src = xt_a if l % 2 == 0 else xt_b
            dst = xt_b if l % 2 == 0 else xt_a
            last = l == 2

            lp = tc.alloc_tile_pool(name=f"lay{l}", bufs=1)
            w_stack = lp.tile([128, NBLK * 176], F32)   # [(g2,hc), (blk,kk,j)]
            blkp = tc.alloc_tile_pool(name=f"blk{l}", bufs=2)
            zp = tc.alloc_tile_pool(name=f"z{l}", bufs=1)
            psA = tc.alloc_tile_pool(name=f"psA{l}", bufs=2, space="PSUM")
            psB = tc.alloc_tile_pool(name=f"psB{l}", bufs=1, space="PSUM")

            for blk in range(NBLK):
                c0 = blk * WBLK
                # --- projections ---
                xt_c = blkp.tile([DK, WBLK], F32, tag="xt")
                nc.sync.dma_start(out=xt_c[:], in_=src[0:DK, c0:c0 + WBLK])
                mmL = psA.tile([68, WBLK], F32, tag="mmL")
                nc.tensor.matmul(mmL[:], wl_t[l][:], xt_c[:], start=True, stop=True)
                mmR = psA.tile([64, WBLK], F32, tag="mmR")
                nc.tensor.matmul(mmR[:], wr_t[l][:], xt_c[:], start=True, stop=True)
                xla = blkp.tile([128, WBLK], BF16, tag="xla")
                nc.vector.tensor_scalar_mul(xla[0:64, :], mmL[0:64, :], aa_t[l][:])
                nc.sync.dma_start(out=xla[64:128, 0:330], in_=xla[0:64, 22:352])
                xra = blkp.tile([128, WBLK], BF16, tag="xra")
                nc.vector.tensor_scalar_mul(xra[0:64, :], mmR[:], aa_t[l][:])
                nc.sync.dma_start(out=xra[64:128, 0:330], in_=xra[0:64, 22:352])
                xlt = blkp.tile([128, WBLK], F32, tag="xlt")
                nc.vector.tensor_copy(out=xlt[0:64, :], in_=mmL[0:64, :])
                nc.sync.dma_start(out=xlt[64:128, 0:330], in_=xlt[0:64, 22:352])
                alq = blkp.tile([68, WBLK], BF16, tag="alq")
                nc.vector.tensor_copy(out=alq[64:68, :], in_=mmL[64:68, :])

                # --- pairwise scores, 128-partition (2 graphs per row-half) ---
                p2 = blkp.tile([128, 3872], BF16, tag="p2")
                in0 = xla[:].rearrange("p (k s) -> p k s", s=44)[:, :, 0:NPG] \
                    .unsqueeze(3).broadcast_to([128, 8, NPG, NPG])
                in1 = xra[:].rearrange("p (k s) -> p k s", s=44)[:, :, 0:NPG] \
                    .unsqueeze(2).broadcast_to([128, 8, NPG, NPG])
                nc.vector.tensor_tensor(out=p2[:], in0=in0, in1=in1, op=OP.add)
                nc.scalar.activation(out=p2[:], in_=p2[:], func=AF.Abs)

                ea = blkp.tile([128, 3872], F32, tag="ea")
                for grp in range(2):
                    sps = psB.tile([128, 4, 512], F32, tag="sps")
                    for gg in range(8):
                        k = grp * 4 + gg // 2
                        g2 = gg % 2
                        h0 = g2 * 64
                        out_ap = sps[h0:h0 + 64, gg // 2, 0:E1]
                        nc.tensor.matmul(out_ap, sgn_t[l][h0:h0 + 64, :],
                                         p2[h0:h0 + 64, k * E1:(k + 1) * E1],
                                         start=True, stop=False)
                    for gg in range(8):
                        k = grp * 4 + gg // 2
                        g2 = gg % 2
                        h0 = g2 * 64
                        out_ap = sps[h0:h0 + 64, gg // 2, 0:E1]
                        cg = k * 44 + g2 * NPG
                        alv = alq[64:68, cg:cg + NPG].unsqueeze(2) \
                            .broadcast_to([4, NPG, NPG])
                        nc.tensor.matmul(out_ap, alw_t[l][64:68, :], alv,
                                         start=False, stop=True)
                    nc.scalar.activation(
                        out=ea[:, grp * 1936:(grp + 1) * 1936]
                            .rearrange("p (k e) -> p k e", e=E1),
                        in_=sps[:, :, 0:E1], func=AF.Exp, scale=0.4)
                # ea cols: (kk: 8, e1: 484), graph = blk*16 + 2*kk + g2(row half)

                den = blkp.tile([128, 176], F32, tag="den")
                nc.vector.tensor_reduce(
                    out=den[:],
                    in_=ea[:].rearrange("p (k i j) -> p k j i", i=NPG, j=NPG),
                    axis=AX.X, op=OP.add)
                rden = blkp.tile([128, 176], F32, tag="rden")
                nc.vector.reciprocal(out=rden[:], in_=den[:])

                if last:
                    alp = zp.tile([128, 3872], F32, tag="alp")
                    rdv = rden[:].rearrange("p (k j) -> p k j", j=NPG) \
                        .unsqueeze(2).broadcast_to([128, 8, NPG, NPG])
                    nc.vector.tensor_tensor(out=alp[:], in0=ea[:], in1=rdv, op=OP.mult)
                    for g2 in range(2):
                        for h in range(4):
                            r = g2 * 64 + h * 16
                            in_a = alp[r:r + 1, :].rearrange("p (k e) -> p k e", e=E1)
                            out_a = bass.AP(alphat_d,
                                            h * EC + (blk * 16 + g2) * E1,
                                            [[2 * E1, 8], [1, E1]])
                            nc.sync.dma_start(out=out_a, in_=in_a)

                z = zp.tile([128, 3872], F32, tag="z")
                xltv = xlt[:].rearrange("p (k s) -> p k s", s=44)[:, :, 0:NPG] \
                    .unsqueeze(3).broadcast_to([128, 8, NPG, NPG])
                nc.vector.tensor_tensor(out=z[:], in0=xltv, in1=ea[:], op=OP.mult)
                wsl = w_stack[:, blk * 176:(blk + 1) * 176]
                nc.vector.tensor_reduce(
                    out=wsl,
                    in_=z[:].rearrange("p (k i j) -> p k j i", i=NPG, j=NPG),
                    axis=AX.X, op=OP.add)
                nc.vector.tensor_tensor(out=wsl, in0=wsl, in1=rden[:], op=OP.mult)
                if last:
                    nc.vector.tensor_scalar_add(wsl, wsl, bias_t[l][:])
                else:
                    nc.vector.tensor_scalar(out=wsl, in0=wsl, scalar1=bias_t[l][:],
                                            scalar2=0.0, op0=OP.add, op1=OP.max)

            psB.release()
            psA.release()
            zp.release()
            blkp.release()

            # --- GraphNorm over w_stack [128, (blk,kk,j)] ---
            gnp = tc.alloc_tile_pool(name=f"gn{l}", bufs=1)
            NCG = NBLK * 8  # 256 graph-cols (pairs of graphs per row-half)
            wv = w_stack[:].rearrange("p (G j) -> p G j", j=NPG)
            mean = gnp.tile([128, NCG], F32)
            nc.vector.tensor_reduce(out=mean[:], in_=wv, axis=AX.X, op=OP.add)
            mmean = gnp.tile([128, NCG], F32)
            nc.vector.tensor_scalar_mul(mmean[:], mean[:], gnm_t[l][:])
            nc.vector.tensor_scalar_mul(mmean[:], mmean[:], 1.0 / NPG)
            cent = gnp.tile([128, NBLK * 176], F32)
            nc.vector.tensor_tensor(
                out=cent[:].rearrange("p (G j) -> p G j", j=NPG), in0=wv,
                in1=mmean[:].unsqueeze(2).broadcast_to([128, NCG, NPG]),
                op=OP.subtract)
            sq = gnp.tile([128, NBLK * 176], F32)
            nc.vector.tensor_tensor(out=sq[:], in0=cent[:], in1=cent[:], op=OP.mult)
            var = gnp.tile([128, NCG], F32)
            nc.vector.tensor_reduce(
                out=var[:], in_=sq[:].rearrange("p (G j) -> p G j", j=NPG),
                axis=AX.X, op=OP.add)
            nc.vector.tensor_scalar(out=var[:], in0=var[:], scalar1=1.0 / NPG,
                                    scalar2=EPS_GN, op0=OP.mult, op1=OP.add)
            sd = gnp.tile([128, NCG], F32)
            nc.scalar.activation(out=sd[:], in_=var[:], func=AF.Sqrt)
            inv = gnp.tile([128, NCG], F32)
            nc.vector.reciprocal(out=inv[:], in_=sd[:])
            winv = gnp.tile([128, NCG], F32)
            nc.vector.tensor_scalar_mul(winv[:], inv[:], gnw_t[l][:])
            gout = gnp.tile([128, NBLK * 176], F32)
            nc.vector.tensor_tensor(
                out=gout[:].rearrange("p (G j) -> p G j", j=NPG),
                in0=cent[:].rearrange("p (G j) -> p G j", j=NPG),
                in1=winv[:].unsqueeze(2).broadcast_to([128, NCG, NPG]), op=OP.mult)
            nc.vector.tensor_scalar_add(gout[:], gout[:], gnb_t[l][:])

            if not last:
                # XT-next: node col = (2*G + g2)*22 + j = G*44 + g2*22 + j
                for g2 in range(2):
                    in_g = gout[g2 * 64:g2 * 64 + 64, :] \
                        .rearrange("p (G j) -> p G j", j=NPG)
                    out_g = bass.AP(dst.tensor, dst.offset + g2 * NPG,
                                    [[NNODE, 64], [44, NCG], [1, NPG]])
                    nc.sync.dma_start(out=out_g, in_=in_g)
            else:
                pooled_t = gnp.tile([128, NCG], F32)
                nc.vector.tensor_reduce(
                    out=pooled_t[:], in_=gout[:].rearrange("p (G j) -> p G j", j=NPG),
                    axis=AX.X, op=OP.add)
                nc.vector.tensor_scalar_mul(pooled_t[:], pooled_t[:], 1.0 / NPG)
                nc.sync.dma_start(out=pooledt_d[:], in_=pooled_t[:])
                with tc.tile_pool(name="ops", bufs=1, space="PSUM") as opsps:
                    osb = gnp.tile([2, 512], F32)
                    for g2 in range(2):
                        ops = opsps.tile([2, NCG], F32, tag=f"o{g2}")
                        nc.tensor.matmul(ops[:], linw_t[g2 * 64:g2 * 64 + 64, :],
                                         pooled_t[g2 * 64:g2 * 64 + 64, :],
                                         start=True, stop=True)
                        nc.vector.tensor_scalar_add(
                            osb[:, g2 * NCG:(g2 + 1) * NCG], ops[:], linb_t[:])
                    nc.sync.dma_start(out=ot_d[:], in_=osb[:])
            gnp.release()
            lp.release()

        consts.release()
        dram.release()

    nc.finalize()
    return nc


def _preprocess(inputs):
    import ml_dtypes
    x = np.ascontiguousarray(np.asarray(inputs["x"], np.float32))
    common = {}
    for li, l in enumerate(("1", "2", "3")):
        Wl = np.asarray(inputs[f"Wl{l}"], np.float32)
        bl = np.asarray(inputs[f"bl{l}"], np.float32)
        Wr = np.asarray(inputs[f"Wr{l}"], np.float32)
        br = np.asarray(inputs[f"br{l}"], np.float32)
        att = np.asarray(inputs[f"att{l}"], np.float32)   # [H, C]
        WlA = np.einsum("dhc,hc->dh", Wl.reshape(-1, H, C), att)
        blA = np.einsum("hc,hc->h", bl.reshape(H, C), att)
        common[f"wl{li}"] = np.concatenate(
            [np.vstack([Wl, bl[None, :]]),
             np.vstack([WlA, blA[None, :]])], axis=1).astype(np.float32)
        common[f"wr{li}"] = np.vstack([Wr, br[None, :]]).astype(np.float32)
        sgn = np.zeros((64, 64), np.float32)
        s = np.sign(att)
        for h in range(H):
            sgn[h * C:(h + 1) * C, h * C:(h + 1) * C] = s[h][:, None]
        common[f"sgn{li}"] = np.tile(sgn, (2, 1)).astype(ml_dtypes.bfloat16)
        alw = np.zeros((68, 64), np.float32)
        for h in range(H):
            alw[64 + h, h * C:(h + 1) * C] = 1.5
        common[f"alw{li}"] = alw.astype(ml_dtypes.bfloat16)
        common[f"aa{li}"] = np.abs(att).reshape(64, 1).astype(np.float32)
        common[f"bias{li}"] = np.tile(np.asarray(inputs[f"bias{l}"], np.float32), 2)[:, None].copy()
        common[f"gnw{li}"] = np.tile(np.asarray(inputs[f"gnw{l}"], np.float32), 2)[:, None].copy()
        common[f"gnb{li}"] = np.tile(np.asarray(inputs[f"gnb{l}"], np.float32), 2)[:, None].copy()
        common[f"gnm{li}"] = np.tile(np.asarray(inputs[f"gnm{l}"], np.float32), 2)[:, None].copy()
    common["linw"] = np.tile(np.asarray(inputs["linW"], np.float32), (2, 1)).copy()
    common["linb"] = np.asarray(inputs["linb"], np.float32).reshape(2, 1).copy()
    common["ident"] = np.eye(128, dtype=np.float32)
    in_maps = []
    for c in range(N_CORES):
        m = dict(common)
        m["x"] = x[c * NNODE:(c + 1) * NNODE].copy()
        in_maps.append(m)
    return in_maps


def kernel(**inputs):
    from concourse.bass_utils import run_bass_kernel_spmd

    if "nc" not in _PROG:
        _PROG["nc"] = _build_program()
    nc = _PROG["nc"]
    in_maps = _preprocess(inputs)
    res = run_bass_kernel_spmd(nc, in_maps, list(range(N_CORES))).results

    o = np.empty((4096, 2), np.float32)
    pooled = np.empty((4096, F1), np.float32)
    alpha3 = np.empty((4096 * E1, H), np.float32)
    bg = np.arange(256)
    for c in range(N_CORES):
        r = res[c]
        alpha3[c * EC:(c + 1) * EC] = r["alphat"].T
        pt = r["pooledt"]
        ot = r["ot"]
        for g2 in range(2):
            g = c * NG + 2 * bg + g2
            pooled[g] = pt[g2 * 64:(g2 + 1) * 64, :][:, bg].T
            o[g] = ot[:, g2 * 256 + bg].T
    return (o, pooled, alpha3)


# revision 7
# speedup vs baseline: 644.6928x; 541.7055x over previous
"""GATv2 x3 + GraphNorm + mean-pool + linear on 8 Trainium2 cores.

Structure exploited: 4096 disjoint fully-connected 22-node graphs.
Sharding: 512 graphs per core (data parallel); weights replicated.
Outputs are written feature-major (alphaT [4,E], pooledT, oT) and
transposed/reordered on the host during unsharding.
"""
import numpy as np

N_CORES = 8
NPG = 22
E1 = NPG * NPG            # 484
H, C, F1 = 4, 16, 64
NG = 4096 // N_CORES      # 512 graphs/core
NNODE = NG * NPG          # 11264 node cols/core
GBLK = 16                 # graphs per block
NBLK = NG // GBLK         # 32
WBLK = GBLK * NPG         # 352 node cols/block
EBLK = GBLK * E1          # 7744 pair cols/block
EC = NG * E1              # 247808 edges/core
EPS_GN = 1e-5

_PROG = {}


def _build_program():
    import concourse.bass as bass
    import concourse.tile as tile
    import concourse.mybir as mybir
    from concourse import bacc

    F32 = mybir.dt.float32
    BF16 = mybir.dt.bfloat16
    AF = mybir.ActivationFunctionType
    OP = mybir.AluOpType
    AX = mybir.AxisListType

    nc = bacc.Bacc(None, target_bir_lowering=False)

    x_d = nc.dram_tensor("x", [NNODE, 22], F32, kind="ExternalInput")
    wl_d, wr_d, sgn_d, aa_d, bias_d, gnw_d, gnb_d, gnm_d = [], [], [], [], [], [], [], []
    alw_d = []
    for l in range(3):
        D = 22 if l == 0 else 64
        wl_d.append(nc.dram_tensor(f"wl{l}", [D + 1, 68], F32, kind="ExternalInput"))
        wr_d.append(nc.dram_tensor(f"wr{l}", [D + 1, 64], F32, kind="ExternalInput"))
        sgn_d.append(nc.dram_tensor(f"sgn{l}", [128, 64], BF16, kind="ExternalInput"))
        alw_d.append(nc.dram_tensor(f"alw{l}", [68, 64], BF16, kind="ExternalInput"))
        aa_d.append(nc.dram_tensor(f"aa{l}", [64, 1], F32, kind="ExternalInput"))
        bias_d.append(nc.dram_tensor(f"bias{l}", [128, 1], F32, kind="ExternalInput"))
        gnw_d.append(nc.dram_tensor(f"gnw{l}", [128, 1], F32, kind="ExternalInput"))
        gnb_d.append(nc.dram_tensor(f"gnb{l}", [128, 1], F32, kind="ExternalInput"))
        gnm_d.append(nc.dram_tensor(f"gnm{l}", [128, 1], F32, kind="ExternalInput"))
    linw_d = nc.dram_tensor("linw", [128, 2], F32, kind="ExternalInput")
    linb_d = nc.dram_tensor("linb", [2, 1], F32, kind="ExternalInput")
    ident_d = nc.dram_tensor("ident", [128, 128], F32, kind="ExternalInput")

    alphat_d = nc.dram_tensor("alphat", [4, EC], F32, kind="ExternalOutput")
    pooledt_d = nc.dram_tensor("pooledt", [128, 256], F32, kind="ExternalOutput")
    ot_d = nc.dram_tensor("ot", [2, 512], F32, kind="ExternalOutput")

    with tile.TileContext(nc) as tc:
        dram = tc.alloc_tile_pool(name="dram", bufs=1, space="DRAM")
        xt_a = dram.tile([65, NNODE], F32)
        xt_b = dram.tile([65, NNODE], F32)

        consts = tc.alloc_tile_pool(name="consts", bufs=1)
        wl_t, wr_t, sgn_t, aa_t, bias_t, gnw_t, gnb_t, gnm_t = [], [], [], [], [], [], [], []
        alw_t = []
        for l in range(3):
            D = 22 if l == 0 else 64
            t = consts.tile([D + 1, 68], F32, tag=f"wl{l}")
            nc.sync.dma_start(out=t[:], in_=wl_d[l][:]); wl_t.append(t)
            t = consts.tile([D + 1, 64], F32, tag=f"wr{l}")
            nc.sync.dma_start(out=t[:], in_=wr_d[l][:]); wr_t.append(t)
            t = consts.tile([128, 64], BF16, tag=f"sgn{l}")
            nc.sync.dma_start(out=t[:], in_=sgn_d[l][:]); sgn_t.append(t)
            t = consts.tile([68, 64], BF16, tag=f"alw{l}")
            nc.sync.dma_start(out=t[:], in_=alw_d[l][:]); alw_t.append(t)
            for lst, dd, tg in ((aa_t, aa_d, "aa"), (bias_t, bias_d, "bias"),
                                (gnw_t, gnw_d, "gnw"), (gnb_t, gnb_d, "gnb"),
                                (gnm_t, gnm_d, "gnm")):
                shp = [64, 1] if tg == "aa" else [128, 1]
                t = consts.tile(shp, F32, tag=f"{tg}{l}")
                nc.sync.dma_start(out=t[:], in_=dd[l][:]); lst.append(t)
        linw_t = consts.tile([128, 2], F32)
        nc.sync.dma_start(out=linw_t[:], in_=linw_d[:])
        linb_t = consts.tile([2, 1], F32)
        nc.sync.dma_start(out=linb_t[:], in_=linb_d[:])
        ident_t = consts.tile([128, 128], F32)
        nc.sync.dma_start(out=ident_t[:], in_=ident_d[:])

        # ---- Stage 0: XT1 = x^T into xt_a rows 0:22; ones rows ----
        with tc.tile_pool(name="tr_sb", bufs=3) as trp, \
             tc.tile_pool(name="tr_ps", bufs=3, space="PSUM") as trps:
            onesrow = trp.tile([1, 1408], F32)
            nc.vector.memset(onesrow[:], 1.0)
            for t8 in range(8):
                sl = slice(t8 * 1408, (t8 + 1) * 1408)
                nc.sync.dma_start(out=xt_a[22:23, sl], in_=onesrow[:])
                nc.sync.dma_start(out=xt_a[64:65, sl], in_=onesrow[:])
                nc.sync.dma_start(out=xt_b[64:65, sl], in_=onesrow[:])
            for t in range(NNODE // 128):
                xm = trp.tile([128, 22], F32, tag="xm")
                nc.sync.dma_start(out=xm[:], in_=x_d[t * 128:(t + 1) * 128, :])
                pt = trps.tile([22, 128], F32, tag="pt")
                nc.tensor.transpose(pt[:], xm[:], ident_t[:])
                st = trp.tile([22, 128], F32, tag="st")
                nc.vector.tensor_copy(out=st[:], in_=pt[:])
                nc.sync.dma_start(out=xt_a[0:22, t * 128:(t + 1) * 128], in_=st[:])

        # ---- Layers ----
        for l in range(3):
            D = 22 if l == 0 else 64
            DK = D + 1
            src = xt_a if l % 2 == 0 else xt_b
            dst = xt_b if l % 2 == 0 else xt_a
            last = l == 2

            lp = tc.alloc_tile_pool(name=f"lay{l}", bufs=1)
            w_stack = lp.tile([128, NBLK * 176], F32)   # [(g2,hc), (blk,kk,j)]
            blkp = tc.alloc_tile_pool(name=f"blk{l}", bufs=2)
            zp = tc.alloc_tile_pool(name=f"z{l}", bufs=1)
            psA = tc.alloc_tile_pool(name=f"psA{l}", bufs=2, space="PSUM")
            psB = tc.alloc_tile_pool(name=f"psB{l}", bufs=1, space="PSUM")

            for blk in range(NBLK):
                c0 = blk * WBLK
                # --- projections ---
                xt_c = blkp.tile([DK, WBLK], F32, tag="xt")
                nc.sync.dma_start(out=xt_c[:], in_=src[0:DK, c0:c0 + WBLK])
                mmL = psA.tile([68, WBLK], F32, tag="mmL")
                nc.tensor.matmul(mmL[:], wl_t[l][:], xt_c[:], start=True, stop=True)
                mmR = psA.tile([64, WBLK], F32, tag="mmR")
                nc.tensor.matmul(mmR[:], wr_t[l][:], xt_c[:], start=True, stop=True)
                xla = blkp.tile([128, WBLK], F32, tag="xla")
                nc.vector.tensor_scalar_mul(xla[0:64, :], mmL[0:64, :], aa_t[l][:])
                nc.sync.dma_start(out=xla[64:128, 0:330], in_=xla[0:64, 22:352])
                xra = blkp.tile([128, WBLK], F32, tag="xra")
                nc.vector.tensor_scalar_mul(xra[0:64, :], mmR[:], aa_t[l][:])
                nc.sync.dma_start(out=xra[64:128, 0:330], in_=xra[0:64, 22:352])
                xlt = blkp.tile([128, WBLK], F32, tag="xlt")
                nc.vector.tensor_copy(out=xlt[0:64, :], in_=mmL[0:64, :])
                nc.sync.dma_start(out=xlt[64:128, 0:330], in_=xlt[0:64, 22:352])
                alq = blkp.tile([68, WBLK], BF16, tag="alq")
                nc.vector.tensor_copy(out=alq[64:68, :], in_=mmL[64:68, :])

                # --- pairwise scores, 128-partition (2 graphs per row-half) ---
                p2 = blkp.tile([128, 3872], BF16, tag="p2")
                in0 = xla[:].rearrange("p (k s) -> p k s", s=44)[:, :, 0:NPG] \
                    .unsqueeze(3).broadcast_to([128, 8, NPG, NPG])
                in1 = xra[:].rearrange("p (k s) -> p k s", s=44)[:, :, 0:NPG] \
                    .unsqueeze(2).broadcast_to([128, 8, NPG, NPG])
                nc.vector.tensor_tensor(out=p2[:], in0=in0, in1=in1, op=OP.add)
                nc.scalar.activation(out=p2[:], in_=p2[:], func=AF.Abs)

                ea = blkp.tile([128, 3872], F32, tag="ea")
                for grp in range(2):
                    sps = psB.tile([128, 4, 512], F32, tag="sps")
                    for gg in range(8):
                        k = grp * 4 + gg // 2
                        g2 = gg % 2
                        h0 = g2 * 64
                        out_ap = sps[h0:h0 + 64, gg // 2, 0:E1]
                        nc.tensor.matmul(out_ap, sgn_t[l][h0:h0 + 64, :],
                                         p2[h0:h0 + 64, k * E1:(k + 1) * E1],
                                         start=True, stop=False)
                    for gg in range(8):
                        k = grp * 4 + gg // 2
                        g2 = gg % 2
                        h0 = g2 * 64
                        out_ap = sps[h0:h0 + 64, gg // 2, 0:E1]
                        cg = k * 44 + g2 * NPG
                        alv = alq[64:68, cg:cg + NPG].unsqueeze(2) \
                            .broadcast_to([4, NPG, NPG])
                        nc.tensor.matmul(out_ap, alw_t[l][64:68, :], alv,
                                         start=False, stop=True)
                    nc.scalar.activation(
                        out=ea[:, grp * 1936:(grp + 1) * 1936]
                            .rearrange("p (k e) -> p k e", e=E1),
                        in_=sps[:, :, 0:E1], func=AF.Exp, scale=0.4)
                # ea cols: (kk: 8, e1: 484), graph = blk*16 + 2*kk + g2(row half)

                den = blkp.tile([128, 176], F32, tag="den")
                nc.vector.tensor_reduce(
                    out=den[:],
                    in_=ea[:].rearrange("p (k i j) -> p k j i", i=NPG, j=NPG),
                    axis=AX.X, op=OP.add)
                rden = blkp.tile([128, 176], F32, tag="rden")
                nc.vector.reciprocal(out=rden[:], in_=den[:])

                if last:
                    alp = zp.tile([128, 3872], F32, tag="alp")
                    rdv = rden[:].rearrange("p (k j) -> p k j", j=NPG) \
                        .unsqueeze(2).broadcast_to([128, 8, NPG, NPG])
                    nc.vector.tensor_tensor(out=alp[:], in0=ea[:], in1=rdv, op=OP.mult)
                    for g2 in range(2):
                        for h in range(4):
                            r = g2 * 64 + h * 16
                            in_a = alp[r:r + 1, :].rearrange("p (k e) -> p k e", e=E1)
                            out_a = bass.AP(alphat_d,
                                            h * EC + (blk * 16 + g2) * E1,
                                            [[2 * E1, 8], [1, E1]])
                            nc.sync.dma_start(out=out_a, in_=in_a)

                z = zp.tile([128, 3872], F32, tag="z")
                xltv = xlt[:].rearrange("p (k s) -> p k s", s=44)[:, :, 0:NPG] \
                    .unsqueeze(3).broadcast_to([128, 8, NPG, NPG])
                nc.vector.tensor_tensor(out=z[:], in0=xltv, in1=ea[:], op=OP.mult)
                wsl = w_stack[:, blk * 176:(blk + 1) * 176]
                nc.vector.tensor_reduce(
                    out=wsl,
                    in_=z[:].rearrange("p (k i j) -> p k j i", i=NPG, j=NPG),
                    axis=AX.X, op=OP.add)
                nc.vector.tensor_tensor(out=wsl, in0=wsl, in1=rden[:], op=OP.mult)
                if last:
                    nc.vector.tensor_scalar_add(wsl, wsl, bias_t[l][:])
                else:
                    nc.vector.tensor_scalar(out=wsl, in0=wsl, scalar1=bias_t[l][:],
                                            scalar2=0.0, op0=OP.add, op1=OP.max)

            psB.release()
            psA.release()
            zp.release()
            blkp.release()

            # --- GraphNorm over w_stack [128, (blk,kk,j)] ---
            gnp = tc.alloc_tile_pool(name=f"gn{l}", bufs=1)
            NCG = NBLK * 8  # 256 graph-cols (pairs of graphs per row-half)
            wv = w_stack[:].rearrange("p (G j) -> p G j", j=NPG)
            mean = gnp.tile([128, NCG], F32)
            nc.vector.tensor_reduce(out=mean[:], in_=wv, axis=AX.X, op=OP.add)
            mmean = gnp.tile([128, NCG], F32)
            nc.vector.tensor_scalar_mul(mmean[:], mean[:], gnm_t[l][:])
            nc.vector.tensor_scalar_mul(mmean[:], mmean[:], 1.0 / NPG)
            cent = gnp.tile([128, NBLK * 176], F32)
            nc.vector.tensor_tensor(
                out=cent[:].rearrange("p (G j) -> p G j", j=NPG), in0=wv,
                in1=mmean[:].unsqueeze(2).broadcast_to([128, NCG, NPG]),
                op=OP.subtract)
            sq = gnp.tile([128, NBLK * 176], F32)
            nc.vector.tensor_tensor(out=sq[:], in0=cent[:], in1=cent[:], op=OP.mult)
            var = gnp.tile([128, NCG], F32)
            nc.vector.tensor_reduce(
                out=var[:], in_=sq[:].rearrange("p (G j) -> p G j", j=NPG),
                axis=AX.X, op=OP.add)
            nc.vector.tensor_scalar(out=var[:], in0=var[:], scalar1=1.0 / NPG,
                                    scalar2=EPS_GN, op0=OP.mult, op1=OP.add)
            sd = gnp.tile([128, NCG], F32)
            nc.scalar.activation(out=sd[:], in_=var[:], func=AF.Sqrt)
            inv = gnp.tile([128, NCG], F32)
            nc.vector.reciprocal(out=inv[:], in_=sd[:])
            winv = gnp.tile([128, NCG], F32)
            nc.vector.tensor_scalar_mul(winv[:], inv[:], gnw_t[l][:])
            gout = gnp.tile([128, NBLK * 176], F32)
            nc.vector.tensor_tensor(
                out=gout[:].rearrange("p (G j) -> p G j", j=NPG),
                in0=cent[:].rearrange("p (G j) -> p G j", j=NPG),
                in1=winv[:].unsqueeze(2).broadcast_to([128, NCG, NPG]), op=OP.mult)
            nc.vector.tensor_scalar_add(gout[:], gout[:], gnb_t[l][:])

            if not last:
                # XT-next: node col = (2*G + g2)*22 + j = G*44 + g2*22 + j
                for g2 in range(2):
                    in_g = gout[g2 * 64:g2 * 64 + 64, :] \
                        .rearrange("p (G j) -> p G j", j=NPG)
                    out_g = bass.AP(dst.tensor, dst.offset + g2 * NPG,
                                    [[NNODE, 64], [44, NCG], [1, NPG]])
                    nc.sync.dma_start(out=out_g, in_=in_g)
            else:
                pooled_t = gnp.tile([128, NCG], F32)
                nc.vector.tensor_reduce(
                    out=pooled_t[:], in_=gout[:].rearrange("p (G j) -> p G j", j=NPG),
                    axis=AX.X, op=OP.add)
                nc.vector.tensor_scalar_mul(pooled_t[:], pooled_t[:], 1.0 / NPG)
                nc.sync.dma_start(out=pooledt_d[:], in_=pooled_t[:])
                with tc.tile_pool(name="ops", bufs=1, space="PSUM") as opsps:
                    osb = gnp.tile([2, 512], F32)
                    for g2 in range(2):
                        ops = opsps.tile([2, NCG], F32, tag=f"o{g2}")
                        nc.tensor.matmul(ops[:], linw_t[g2 * 64:g2 * 64 + 64, :],
                                         pooled_t[g2 * 64:g2 * 64 + 64, :],
                                         start=True, stop=True)
                        nc.vector.tensor_scalar_add(
                            osb[:, g2 * NCG:(g2 + 1) * NCG], ops[:], linb_t[:])
                    nc.sync.dma_start(out=ot_d[:], in_=osb[:])
            gnp.release()
            lp.release()

        consts.release()
        dram.release()

    nc.finalize()
    return nc


def _preprocess(inputs):
    import ml_dtypes
    x = np.ascontiguousarray(np.asarray(inputs["x"], np.float32))
    common = {}
    for li, l in enumerate(("1", "2", "3")):
        Wl = np.asarray(inputs[f"Wl{l}"], np.float32)
        bl = np.asarray(inputs[f"bl{l}"], np.float32)
        Wr = np.asarray(inputs[f"Wr{l}"], np.float32)
        br = np.asarray(inputs[f"br{l}"], np.float32)
        att = np.asarray(inputs[f"att{l}"], np.float32)   # [H, C]
        WlA = np.einsum("dhc,hc->dh", Wl.reshape(-1, H, C), att)
        blA = np.einsum("hc,hc->h", bl.reshape(H, C), att)
        common[f"wl{li}"] = np.concatenate(
            [np.vstack([Wl, bl[None, :]]),
             np.vstack([WlA, blA[None, :]])], axis=1).astype(np.float32)
        common[f"wr{li}"] = np.vstack([Wr, br[None, :]]).astype(np.float32)
        sgn = np.zeros((64, 64), np.float32)
        s = np.sign(att)
        for h in range(H):
            sgn[h * C:(h + 1) * C, h * C:(h + 1) * C] = s[h][:, None]
        common[f"sgn{li}"] = np.tile(sgn, (2, 1)).astype(ml_dtypes.bfloat16)
        alw = np.zeros((68, 64), np.float32)
        for h in range(H):
            alw[64 + h, h * C:(h + 1) * C] = 1.5
        common[f"alw{li}"] = alw.astype(ml_dtypes.bfloat16)
        common[f"aa{li}"] = np.abs(att).reshape(64, 1).astype(np.float32)
        common[f"bias{li}"] = np.tile(np.asarray(inputs[f"bias{l}"], np.float32), 2)[:, None].copy()
        common[f"gnw{li}"] = np.tile(np.asarray(inputs[f"gnw{l}"], np.float32), 2)[:, None].copy()
        common[f"gnb{li}"] = np.tile(np.asarray(inputs[f"gnb{l}"], np.float32), 2)[:, None].copy()
        common[f"gnm{li}"] = np.tile(np.asarray(inputs[f"gnm{l}"], np.float32), 2)[:, None].copy()
    common["linw"] = np.tile(np.asarray(inputs["linW"], np.float32), (2, 1)).copy()
    common["linb"] = np.asarray(inputs["linb"], np.float32).reshape(2, 1).copy()
    common["ident"] = np.eye(128, dtype=np.float32)
    in_maps = []
    for c in range(N_CORES):
        m = dict(common)
        m["x"] = x[c * NNODE:(c + 1) * NNODE].copy()
        in_maps.append(m)
    return in_maps


def kernel(**inputs):
    from concourse.bass_utils import run_bass_kernel_spmd

    if "nc" not in _PROG:
        _PROG["nc"] = _build_program()
    nc = _PROG["nc"]
    in_maps = _preprocess(inputs)
    res = run_bass_kernel_spmd(nc, in_maps, list(range(N_CORES))).results

    o = np.empty((4096, 2), np.float32)
    pooled = np.empty((4096, F1), np.float32)
    alpha3 = np.empty((4096 * E1, H), np.float32)
    bg = np.arange(256)
    for c in range(N_CORES):
        r = res[c]
        alpha3[c * EC:(c + 1) * EC] = r["alphat"].T
        pt = r["pooledt"]
        ot = r["ot"]
        for g2 in range(2):
            g = c * NG + 2 * bg + g2
            pooled[g] = pt[g2 * 64:(g2 + 1) * 64, :][:, bg].T
            o[g] = ot[:, g2 * 256 + bg].T
    return (o, pooled, alpha3)


# revision 8
# speedup vs baseline: 693.2240x; 1.0753x over previous
"""GATv2 x3 + GraphNorm + mean-pool + linear on 8 Trainium2 cores.

Structure exploited: 4096 disjoint fully-connected 22-node graphs.
Sharding: 512 graphs per core (data parallel); weights replicated.
Outputs are written feature-major (alphaT [4,E], pooledT, oT) and
transposed/reordered on the host during unsharding.
"""
import numpy as np

N_CORES = 8
NPG = 22
E1 = NPG * NPG            # 484
H, C, F1 = 4, 16, 64
NG = 4096 // N_CORES      # 512 graphs/core
NNODE = NG * NPG          # 11264 node cols/core
GBLK = 16                 # graphs per block
NBLK = NG // GBLK         # 32
WBLK = GBLK * NPG         # 352 node cols/block
EBLK = GBLK * E1          # 7744 pair cols/block
EC = NG * E1              # 247808 edges/core
EPS_GN = 1e-5

_PROG = {}


def _build_program():
    import concourse.bass as bass
    import concourse.tile as tile
    import concourse.mybir as mybir
    from concourse import bacc

    F32 = mybir.dt.float32
    BF16 = mybir.dt.bfloat16
    AF = mybir.ActivationFunctionType
    OP = mybir.AluOpType
    AX = mybir.AxisListType

    nc = bacc.Bacc(None, target_bir_lowering=False)

    x_d = nc.dram_tensor("x", [NNODE, 22], F32, kind="ExternalInput")
    wl_d, wr_d, sgn_d, aa_d, bias_d, gnw_d, gnb_d, gnm_d = [], [], [], [], [], [], [], []
    alw_d = []
    for l in range(3):
        D = 22 if l == 0 else 64
        wl_d.append(nc.dram_tensor(f"wl{l}", [D + 1, 68], F32, kind="ExternalInput"))
        wr_d.append(nc.dram_tensor(f"wr{l}", [D + 1, 64], F32, kind="ExternalInput"))
        sgn_d.append(nc.dram_tensor(f"sgn{l}", [128, 64], BF16, kind="ExternalInput"))
        alw_d.append(nc.dram_tensor(f"alw{l}", [68, 64], BF16, kind="ExternalInput"))
        aa_d.append(nc.dram_tensor(f"aa{l}", [64, 1], F32, kind="ExternalInput"))
        bias_d.append(nc.dram_tensor(f"bias{l}", [128, 1], F32, kind="ExternalInput"))
        gnw_d.append(nc.dram_tensor(f"gnw{l}", [128, 1], F32, kind="ExternalInput"))
        gnb_d.append(nc.dram_tensor(f"gnb{l}", [128, 1], F32, kind="ExternalInput"))
        gnm_d.append(nc.dram_tensor(f"gnm{l}", [128, 1], F32, kind="ExternalInput"))
    linw_d = nc.dram_tensor("linw", [128, 2], F32, kind="ExternalInput")
    linb_d = nc.dram_tensor("linb", [2, 1], F32, kind="ExternalInput")
    ident_d = nc.dram_tensor("ident", [128, 128], F32, kind="ExternalInput")

    alphat_d = nc.dram_tensor("alphat", [4, EC], F32, kind="ExternalOutput")
    pooledt_d = nc.dram_tensor("pooledt", [128, 256], F32, kind="ExternalOutput")
    ot_d = nc.dram_tensor("ot", [2, 512], F32, kind="ExternalOutput")

    with tile.TileContext(nc) as tc:
        dram = tc.alloc_tile_pool(name="dram", bufs=1, space="DRAM")
        xt_a = dram.tile([65, NNODE], F32)
        xt_b = dram.tile([65, NNODE], F32)

        consts = tc.alloc_tile_pool(name="consts", bufs=1)
        wl_t, wr_t, sgn_t, aa_t, bias_t, gnw_t, gnb_t, gnm_t = [], [], [], [], [], [], [], []
        alw_t = []
        for l in range(3):
            D = 22 if l == 0 else 64
            t = consts.tile([D + 1, 68], F32, tag=f"wl{l}")
            nc.sync.dma_start(out=t[:], in_=wl_d[l][:]); wl_t.append(t)
            t = consts.tile([D + 1, 64], F32, tag=f"wr{l}")
            nc.sync.dma_start(out=t[:], in_=wr_d[l][:]); wr_t.append(t)
            t = consts.tile([128, 64], BF16, tag=f"sgn{l}")
            nc.sync.dma_start(out=t[:], in_=sgn_d[l][:]); sgn_t.append(t)
            t = consts.tile([68, 64], BF16, tag=f"alw{l}")
            nc.sync.dma_start(out=t[:], in_=alw_d[l][:]); alw_t.append(t)
            for lst, dd, tg in ((aa_t, aa_d, "aa"), (bias_t, bias_d, "bias"),
                                (gnw_t, gnw_d, "gnw"), (gnb_t, gnb_d, "gnb"),
                                (gnm_t, gnm_d, "gnm")):
                shp = [64, 1] if tg == "aa" else [128, 1]
                t = consts.tile(shp, F32, tag=f"{tg}{l}")
                nc.sync.dma_start(out=t[:], in_=dd[l][:]); lst.append(t)
        linw_t = consts.tile([128, 2], F32)
        nc.sync.dma_start(out=linw_t[:], in_=linw_d[:])
        linb_t = consts.tile([2, 1], F32)
        nc.sync.dma_start(out=linb_t[:], in_=linb_d[:])
        ident_t = consts.tile([128, 128], F32)
        nc.sync.dma_start(out=ident_t[:], in_=ident_d[:])

        # ---- Stage 0: XT1 = x^T into xt_a rows 0:22; ones rows ----
        with tc.tile_pool(name="tr_sb", bufs=3) as trp, \
             tc.tile_pool(name="tr_ps", bufs=3, space="PSUM") as trps:
            onesrow = trp.tile([1, 1408], F32)
            nc.vector.memset(onesrow[:], 1.0)
            for t8 in range(8):
                sl = slice(t8 * 1408, (t8 + 1) * 1408)
                nc.sync.dma_start(out=xt_a[22:23, sl], in_=onesrow[:])
                nc.sync.dma_start(out=xt_a[64:65, sl], in_=onesrow[:])
                nc.sync.dma_start(out=xt_b[64:65, sl], in_=onesrow[:])
            for t in range(NNODE // 128):
                xm = trp.tile([128, 22], F32, tag="xm")
                nc.sync.dma_start(out=xm[:], in_=x_d[t * 128:(t + 1) * 128, :])
                pt = trps.tile([22, 128], F32, tag="pt")
                nc.tensor.transpose(pt[:], xm[:], ident_t[:])
                st = trp.tile([22, 128], F32, tag="st")
                nc.vector.tensor_copy(out=st[:], in_=pt[:])
                nc.sync.dma_start(out=xt_a[0:22, t * 128:(t + 1) * 128], in_=st[:])

        # ---- Layers ----
        for l in range(3):
            D = 22 if l == 0 else 64
            DK = D + 1
            src = xt_a if l % 2 == 0 else xt_b
            dst = xt_b if l % 2 == 0 else xt_a
            last = l == 2

            lp = tc.alloc_tile_pool(name=f"lay{l}", bufs=1)
            w_stack = lp.tile([128, NBLK * 176], F32)   # [(g2,hc), (blk,kk,j)]
            blkp = tc.alloc_tile_pool(name=f"blk{l}", bufs=2)
            zp = tc.alloc_tile_pool(name=f"z{l}", bufs=2)
            psA = tc.alloc_tile_pool(name=f"psA{l}", bufs=2, space="PSUM")
            psB = tc.alloc_tile_pool(name=f"psB{l}", bufs=1, space="PSUM")

            for blk in range(NBLK):
                c0 = blk * WBLK
                # --- projections ---
                xt_c = blkp.tile([DK, WBLK], F32, tag="xt")
                nc.sync.dma_start(out=xt_c[:], in_=src[0:DK, c0:c0 + WBLK])
                mmL = psA.tile([68, WBLK], F32, tag="mmL")
                nc.tensor.matmul(mmL[:], wl_t[l][:], xt_c[:], start=True, stop=True)
                mmR = psA.tile([64, WBLK], F32, tag="mmR")
                nc.tensor.matmul(mmR[:], wr_t[l][:], xt_c[:], start=True, stop=True)
                xla = blkp.tile([128, WBLK], F32, tag="xla")
                nc.vector.tensor_scalar_mul(xla[0:64, :], mmL[0:64, :], aa_t[l][:])
                nc.sync.dma_start(out=xla[64:128, 0:330], in_=xla[0:64, 22:352])
                xra = blkp.tile([128, WBLK], F32, tag="xra")
                nc.vector.tensor_scalar_mul(xra[0:64, :], mmR[:], aa_t[l][:])
                nc.sync.dma_start(out=xra[64:128, 0:330], in_=xra[0:64, 22:352])
                xlt = blkp.tile([128, WBLK], F32, tag="xlt")
                nc.vector.tensor_copy(out=xlt[0:64, :], in_=mmL[0:64, :])
                nc.sync.dma_start(out=xlt[64:128, 0:330], in_=xlt[0:64, 22:352])
                alq = blkp.tile([68, WBLK], BF16, tag="alq")
                nc.vector.tensor_copy(out=alq[64:68, :], in_=mmL[64:68, :])

                # --- pairwise scores, 128-partition (2 graphs per row-half) ---
                p2 = blkp.tile([128, 3872], BF16, tag="p2")
                in0 = xla[:].rearrange("p (k s) -> p k s", s=44)[:, :, 0:NPG] \
                    .unsqueeze(3).broadcast_to([128, 8, NPG, NPG])
                in1 = xra[:].rearrange("p (k s) -> p k s", s=44)[:, :, 0:NPG] \
                    .unsqueeze(2).broadcast_to([128, 8, NPG, NPG])
                nc.vector.tensor_tensor(out=p2[:], in0=in0, in1=in1, op=OP.add)
                nc.scalar.activation(out=p2[:], in_=p2[:], func=AF.Abs)

                ea = blkp.tile([128, 3872], F32, tag="ea")
                for grp in range(2):
                    sps = psB.tile([128, 4, 512], F32, tag="sps")
                    for gg in range(8):
                        k = grp * 4 + gg // 2
                        g2 = gg % 2
                        h0 = g2 * 64
                        out_ap = sps[h0:h0 + 64, gg // 2, 0:E1]
                        nc.tensor.matmul(out_ap, sgn_t[l][h0:h0 + 64, :],
                                         p2[h0:h0 + 64, k * E1:(k + 1) * E1],
                                         start=True, stop=False)
                    for gg in range(8):
                        k = grp * 4 + gg // 2
                        g2 = gg % 2
                        h0 = g2 * 64
                        out_ap = sps[h0:h0 + 64, gg // 2, 0:E1]
                        cg = k * 44 + g2 * NPG
                        alv = alq[64:68, cg:cg + NPG].unsqueeze(2) \
                            .broadcast_to([4, NPG, NPG])
                        nc.tensor.matmul(out_ap, alw_t[l][64:68, :], alv,
                                         start=False, stop=True)
                    nc.scalar.activation(
                        out=ea[:, grp * 1936:(grp + 1) * 1936]
                            .rearrange("p (k e) -> p k e", e=E1),
                        in_=sps[:, :, 0:E1], func=AF.Exp, scale=0.4)
                # ea cols: (kk: 8, e1: 484), graph = blk*16 + 2*kk + g2(row half)

                den = blkp.tile([128, 176], F32, tag="den")
                nc.vector.tensor_reduce(
                    out=den[:],
                    in_=ea[:].rearrange("p (k i j) -> p k j i", i=NPG, j=NPG),
                    axis=AX.X, op=OP.add)
                rden = blkp.tile([128, 176], F32, tag="rden")
                nc.vector.reciprocal(out=rden[:], in_=den[:])

                if last:
                    alp = zp.tile([128, 3872], F32, tag="alp")
                    rdv = rden[:].rearrange("p (k j) -> p k j", j=NPG) \
                        .unsqueeze(2).broadcast_to([128, 8, NPG, NPG])
                    nc.vector.tensor_tensor(out=alp[:], in0=ea[:], in1=rdv, op=OP.mult)
                    for g2 in range(2):
                        for h in range(4):
                            r = g2 * 64 + h * 16
                            in_a = alp[r:r + 1, :].rearrange("p (k e) -> p k e", e=E1)
                            out_a = bass.AP(alphat_d,
                                            h * EC + (blk * 16 + g2) * E1,
                                            [[2 * E1, 8], [1, E1]])
                            nc.sync.dma_start(out=out_a, in_=in_a)

                z = zp.tile([128, 3872], F32, tag="z")
                xltv = xlt[:].rearrange("p (k s) -> p k s", s=44)[:, :, 0:NPG] \
                    .unsqueeze(3).broadcast_to([128, 8, NPG, NPG])
                nc.gpsimd.tensor_tensor(out=z[:], in0=xltv, in1=ea[:], op=OP.mult)
                wsl = w_stack[:, blk * 176:(blk + 1) * 176]
                nc.vector.tensor_reduce(
                    out=wsl,
                    in_=z[:].rearrange("p (k i j) -> p k j i", i=NPG, j=NPG),
                    axis=AX.X, op=OP.add)
                nc.vector.tensor_tensor(out=wsl, in0=wsl, in1=rden[:], op=OP.mult)
                if last:
                    nc.vector.tensor_scalar_add(wsl, wsl, bias_t[l][:])
                else:
                    nc.vector.tensor_scalar(out=wsl, in0=wsl, scalar1=bias_t[l][:],
                                            scalar2=0.0, op0=OP.add, op1=OP.max)

            psB.release()
            psA.release()
            zp.release()
            blkp.release()

            # --- GraphNorm over w_stack [128, (blk,kk,j)] ---
            gnp = tc.alloc_tile_pool(name=f"gn{l}", bufs=1)
            NCG = NBLK * 8  # 256 graph-cols (pairs of graphs per row-half)
            wv = w_stack[:].rearrange("p (G j) -> p G j", j=NPG)
            mean = gnp.tile([128, NCG], F32)
            nc.vector.tensor_reduce(out=mean[:], in_=wv, axis=AX.X, op=OP.add)
            mmean = gnp.tile([128, NCG], F32)
            nc.vector.tensor_scalar_mul(mmean[:], mean[:], gnm_t[l][:])
            nc.vector.tensor_scalar_mul(mmean[:], mmean[:], 1.0 / NPG)
            cent = gnp.tile([128, NBLK * 176], F32)
            nc.vector.tensor_tensor(
                out=cent[:].rearrange("p (G j) -> p G j", j=NPG), in0=wv,
                in1=mmean[:].unsqueeze(2).broadcast_to([128, NCG, NPG]),
                op=OP.subtract)
            sq = gnp.tile([128, NBLK * 176], F32)
            nc.vector.tensor_tensor(out=sq[:], in0=cent[:], in1=cent[:], op=OP.mult)
            var = gnp.tile([128, NCG], F32)
            nc.vector.tensor_reduce(
                out=var[:], in_=sq[:].rearrange("p (G j) -> p G j", j=NPG),
                axis=AX.X, op=OP.add)
            nc.vector.tensor_scalar(out=var[:], in0=var[:], scalar1=1.0 / NPG,
                                    scalar2=EPS_GN, op0=OP.mult, op1=OP.add)
            sd = gnp.tile([128, NCG], F32)
            nc.scalar.activation(out=sd[:], in_=var[:], func=AF.Sqrt)
            inv = gnp.tile([128, NCG], F32)
            nc.vector.reciprocal(out=inv[:], in_=sd[:])
            winv = gnp.tile([128, NCG], F32)
            nc.vector.tensor_scalar_mul(winv[:], inv[:], gnw_t[l][:])
            gout = gnp.tile([128, NBLK * 176], F32)
            nc.vector.tensor_tensor(
                out=gout[:].rearrange("p (G j) -> p G j", j=NPG),
                in0=cent[:].rearrange("p (G j) -> p G j", j=NPG),
                in1=winv[:].unsqueeze(2).broadcast_to([128, NCG, NPG]), op=OP.mult)
            nc.vector.tensor_scalar_add(gout[:], gout[:], gnb_t[l][:])

            if not last:
                # XT-next: node col = (2*G + g2)*22 + j = G*44 + g2*22 + j
                for g2 in range(2):
                    in_g = gout[g2 * 64:g2 * 64 + 64, :] \
                        .rearrange("p (G j) -> p G j", j=NPG)
                    out_g = bass.AP(dst.tensor, dst.offset + g2 * NPG,
                                    [[NNODE, 64], [44, NCG], [1, NPG]])
                    nc.sync.dma_start(out=out_g, in_=in_g)
            else:
                pooled_t = gnp.tile([128, NCG], F32)
                nc.vector.tensor_reduce(
                    out=pooled_t[:], in_=gout[:].rearrange("p (G j) -> p G j", j=NPG),
                    axis=AX.X, op=OP.add)
                nc.vector.tensor_scalar_mul(pooled_t[:], pooled_t[:], 1.0 / NPG)
                nc.sync.dma_start(out=pooledt_d[:], in_=pooled_t[:])
                with tc.tile_pool(name="ops", bufs=1, space="PSUM") as opsps:
                    osb = gnp.tile([2, 512], F32)
                    for g2 in range(2):
                        ops = opsps.tile([2, NCG], F32, tag=f"o{g2}")
                        nc.tensor.matmul(ops[:], linw_t[g2 * 64:g2 * 64 + 64, :],
                                         pooled_t[g2 * 64:g2 * 64 + 64, :],
                                         start=True, stop=True)
                        nc.vector.tensor_scalar_add(
                            osb[:, g2 * NCG:(g2 + 1) * NCG], ops[:], linb_t[:])
                    nc.sync.dma_start(out=ot_d[:], in_=osb[:])
            gnp.release()
            lp.release()

        consts.release()
        dram.release()

    nc.finalize()
    return nc


def _preprocess(inputs):
    import ml_dtypes
    x = np.ascontiguousarray(np.asarray(inputs["x"], np.float32))
    common = {}
    for li, l in enumerate(("1", "2", "3")):
        Wl = np.asarray(inputs[f"Wl{l}"], np.float32)
        bl = np.asarray(inputs[f"bl{l}"], np.float32)
        Wr = np.asarray(inputs[f"Wr{l}"], np.float32)
        br = np.asarray(inputs[f"br{l}"], np.float32)
        att = np.asarray(inputs[f"att{l}"], np.float32)   # [H, C]
        WlA = np.einsum("dhc,hc->dh", Wl.reshape(-1, H, C), att)
        blA = np.einsum("hc,hc->h", bl.reshape(H, C), att)
        common[f"wl{li}"] = np.concatenate(
            [np.vstack([Wl, bl[None, :]]),
             np.vstack([WlA, blA[None, :]])], axis=1).astype(np.float32)
        common[f"wr{li}"] = np.vstack([Wr, br[None, :]]).astype(np.float32)
        sgn = np.zeros((64, 64), np.float32)
        s = np.sign(att)
        for h in range(H):
            sgn[h * C:(h + 1) * C, h * C:(h + 1) * C] = s[h][:, None]
        common[f"sgn{li}"] = np.tile(sgn, (2, 1)).astype(ml_dtypes.bfloat16)
        alw = np.zeros((68, 64), np.float32)
        for h in range(H):
            alw[64 + h, h * C:(h + 1) * C] = 1.5
        common[f"alw{li}"] = alw.astype(ml_dtypes.bfloat16)
        common[f"aa{li}"] = np.abs(att).reshape(64, 1).astype(np.float32)
        common[f"bias{li}"] = np.tile(np.asarray(inputs[f"bias{l}"], np.float32), 2)[:, None].copy()
        common[f"gnw{li}"] = np.tile(np.asarray(inputs[f"gnw{l}"], np.float32), 2)[:, None].copy()
        common[f"gnb{li}"] = np.tile(np.asarray(inputs[f"gnb{l}"], np.float32), 2)[:, None].copy()
        common[f"gnm{li}"] = np.tile(np.asarray(inputs[f"gnm{l}"], np.float32), 2)[:, None].copy()
    common["linw"] = np.tile(np.asarray(inputs["linW"], np.float32), (2, 1)).copy()
    common["linb"] = np.asarray(inputs["linb"], np.float32).reshape(2, 1).copy()
    common["ident"] = np.eye(128, dtype=np.float32)
    in_maps = []
    for c in range(N_CORES):
        m = dict(common)
        m["x"] = x[c * NNODE:(c + 1) * NNODE].copy()
        in_maps.append(m)
    return in_maps


def kernel(**inputs):
    from concourse.bass_utils import run_bass_kernel_spmd

    if "nc" not in _PROG:
        _PROG["nc"] = _build_program()
    nc = _PROG["nc"]
    in_maps = _preprocess(inputs)
    res = run_bass_kernel_spmd(nc, in_maps, list(range(N_CORES))).results

    o = np.empty((4096, 2), np.float32)
    pooled = np.empty((4096, F1), np.float32)
    alpha3 = np.empty((4096 * E1, H), np.float32)
    bg = np.arange(256)
    for c in range(N_CORES):
        r = res[c]
        alpha3[c * EC:(c + 1) * EC] = r["alphat"].T
        pt = r["pooledt"]
        ot = r["ot"]
        for g2 in range(2):
            g = c * NG + 2 * bg + g2
            pooled[g] = pt[g2 * 64:(g2 + 1) * 64, :][:, bg].T
            o[g] = ot[:, g2 * 256 + bg].T
    return (o, pooled, alpha3)


# revision 12
# speedup vs baseline: 775.9222x; 1.1193x over previous
"""GATv2 x3 + GraphNorm + mean-pool + linear on 8 Trainium2 cores.

Structure exploited: 4096 disjoint fully-connected 22-node graphs.
Sharding: 512 graphs per core (data parallel); weights replicated.
Outputs are written feature-major (alphaT [4,E], pooledT, oT) and
transposed/reordered on the host during unsharding.
"""
import numpy as np

N_CORES = 8
NPG = 22
E1 = NPG * NPG            # 484
H, C, F1 = 4, 16, 64
NG = 4096 // N_CORES      # 512 graphs/core
NNODE = NG * NPG          # 11264 node cols/core
GBLK = 16                 # graphs per block
NBLK = NG // GBLK         # 32
WBLK = GBLK * NPG         # 352 node cols/block
EBLK = GBLK * E1          # 7744 pair cols/block
EC = NG * E1              # 247808 edges/core
EPS_GN = 1e-5

_PROG = {}


def _build_program():
    import concourse.bass as bass
    import concourse.tile as tile
    import concourse.mybir as mybir
    from concourse import bacc

    F32 = mybir.dt.float32
    BF16 = mybir.dt.bfloat16
    AF = mybir.ActivationFunctionType
    OP = mybir.AluOpType
    AX = mybir.AxisListType

    nc = bacc.Bacc(None, target_bir_lowering=False)

    x_d = nc.dram_tensor("x", [NNODE, 22], F32, kind="ExternalInput")
    wl_d, wr_d, sgn_d, aa_d, bias_d, gnw_d, gnb_d, gnm_d = [], [], [], [], [], [], [], []
    alw_d = []
    for l in range(3):
        D = 22 if l == 0 else 64
        wl_d.append(nc.dram_tensor(f"wl{l}", [D + 1, 68], F32, kind="ExternalInput"))
        wr_d.append(nc.dram_tensor(f"wr{l}", [D + 1, 64], F32, kind="ExternalInput"))
        sgn_d.append(nc.dram_tensor(f"sgn{l}", [128, 64], BF16, kind="ExternalInput"))
        alw_d.append(nc.dram_tensor(f"alw{l}", [68, 64], BF16, kind="ExternalInput"))
        aa_d.append(nc.dram_tensor(f"aa{l}", [64, 1], F32, kind="ExternalInput"))
        bias_d.append(nc.dram_tensor(f"bias{l}", [128, 1], F32, kind="ExternalInput"))
        gnw_d.append(nc.dram_tensor(f"gnw{l}", [128, 1], F32, kind="ExternalInput"))
        gnb_d.append(nc.dram_tensor(f"gnb{l}", [128, 1], F32, kind="ExternalInput"))
        gnm_d.append(nc.dram_tensor(f"gnm{l}", [128, 1], F32, kind="ExternalInput"))
    linw_d = nc.dram_tensor("linw", [128, 2], F32, kind="ExternalInput")
    linb_d = nc.dram_tensor("linb", [2, 1], F32, kind="ExternalInput")
    ident_d = nc.dram_tensor("ident", [128, 128], F32, kind="ExternalInput")

    alphat_d = nc.dram_tensor("alphat", [4, EC], F32, kind="ExternalOutput")
    pooledt_d = nc.dram_tensor("pooledt", [128, 256], F32, kind="ExternalOutput")
    ot_d = nc.dram_tensor("ot", [2, 512], F32, kind="ExternalOutput")

    with tile.TileContext(nc) as tc:
        dram = tc.alloc_tile_pool(name="dram", bufs=1, space="DRAM")
        xt_a = dram.tile([65, NNODE], F32)
        xt_b = dram.tile([65, NNODE], F32)

        consts = tc.alloc_tile_pool(name="consts", bufs=1)
        wl_t, wr_t, sgn_t, aa_t, bias_t, gnw_t, gnb_t, gnm_t = [], [], [], [], [], [], [], []
        alw_t = []
        for l in range(3):
            D = 22 if l == 0 else 64
            t = consts.tile([D + 1, 68], F32, tag=f"wl{l}")
            nc.sync.dma_start(out=t[:], in_=wl_d[l][:]); wl_t.append(t)
            t = consts.tile([D + 1, 64], F32, tag=f"wr{l}")
            nc.sync.dma_start(out=t[:], in_=wr_d[l][:]); wr_t.append(t)
            t = consts.tile([128, 64], BF16, tag=f"sgn{l}")
            nc.sync.dma_start(out=t[:], in_=sgn_d[l][:]); sgn_t.append(t)
            t = consts.tile([68, 64], BF16, tag=f"alw{l}")
            nc.sync.dma_start(out=t[:], in_=alw_d[l][:]); alw_t.append(t)
            for lst, dd, tg in ((aa_t, aa_d, "aa"), (bias_t, bias_d, "bias"),
                                (gnw_t, gnw_d, "gnw"), (gnb_t, gnb_d, "gnb"),
                                (gnm_t, gnm_d, "gnm")):
                shp = [64, 1] if tg == "aa" else [128, 1]
                t = consts.tile(shp, F32, tag=f"{tg}{l}")
                nc.sync.dma_start(out=t[:], in_=dd[l][:]); lst.append(t)
        linw_t = consts.tile([128, 2], F32)
        nc.sync.dma_start(out=linw_t[:], in_=linw_d[:])
        linb_t = consts.tile([2, 1], F32)
        nc.sync.dma_start(out=linb_t[:], in_=linb_d[:])
        ident_t = consts.tile([128, 128], F32)
        nc.sync.dma_start(out=ident_t[:], in_=ident_d[:])

        # ---- Stage 0: XT1 = x^T into xt_a rows 0:22; ones rows ----
        with tc.tile_pool(name="tr_sb", bufs=3) as trp, \
             tc.tile_pool(name="tr_ps", bufs=3, space="PSUM") as trps:
            onesrow = trp.tile([1, 1408], F32)
            nc.vector.memset(onesrow[:], 1.0)
            for t8 in range(8):
                sl = slice(t8 * 1408, (t8 + 1) * 1408)
                nc.sync.dma_start(out=xt_a[22:23, sl], in_=onesrow[:])
                nc.sync.dma_start(out=xt_a[64:65, sl], in_=onesrow[:])
                nc.sync.dma_start(out=xt_b[64:65, sl], in_=onesrow[:])
            for t in range(NNODE // 128):
                xm = trp.tile([128, 22], F32, tag="xm")
                nc.sync.dma_start(out=xm[:], in_=x_d[t * 128:(t + 1) * 128, :])
                pt = trps.tile([22, 128], F32, tag="pt")
                nc.tensor.transpose(pt[:], xm[:], ident_t[:])
                st = trp.tile([22, 128], F32, tag="st")
                nc.vector.tensor_copy(out=st[:], in_=pt[:])
                nc.sync.dma_start(out=xt_a[0:22, t * 128:(t + 1) * 128], in_=st[:])

        # ---- Layers ----
        for l in range(3):
            D = 22 if l == 0 else 64
            DK = D + 1
            src = xt_a if l % 2 == 0 else xt_b
            dst = xt_b if l % 2 == 0 else xt_a
            last = l == 2

            lp = tc.alloc_tile_pool(name=f"lay{l}", bufs=1)
            w_stack = lp.tile([128, NBLK * 176], F32)   # [(g2,hc), (blk,kk,j)]
            blkp = tc.alloc_tile_pool(name=f"blk{l}", bufs=3)
            zp = tc.alloc_tile_pool(name=f"z{l}", bufs=2)
            psA = tc.alloc_tile_pool(name=f"psA{l}", bufs=2, space="PSUM")
            psB = tc.alloc_tile_pool(name=f"psB{l}", bufs=1, space="PSUM")

            for blk in range(NBLK):
                c0 = blk * WBLK
                # --- projections ---
                xt_c = blkp.tile([DK, WBLK], F32, tag="xt")
                nc.sync.dma_start(out=xt_c[:], in_=src[0:DK, c0:c0 + WBLK])
                mmL = psA.tile([68, WBLK], F32, tag="mmL")
                nc.tensor.matmul(mmL[:], wl_t[l][:], xt_c[:], start=True, stop=True)
                mmR = psA.tile([64, WBLK], F32, tag="mmR")
                nc.tensor.matmul(mmR[:], wr_t[l][:], xt_c[:], start=True, stop=True)
                xla = blkp.tile([128, WBLK], F32, tag="xla")
                nc.vector.tensor_scalar_mul(xla[0:64, :], mmL[0:64, :], aa_t[l][:])
                nc.sync.dma_start(out=xla[64:128, 0:330], in_=xla[0:64, 22:352])
                xra = blkp.tile([128, WBLK], F32, tag="xra")
                nc.vector.tensor_scalar_mul(xra[0:64, :], mmR[:], aa_t[l][:])
                nc.sync.dma_start(out=xra[64:128, 0:330], in_=xra[0:64, 22:352])
                xlt = blkp.tile([128, WBLK], F32, tag="xlt")
                nc.vector.tensor_copy(out=xlt[0:64, :], in_=mmL[0:64, :])
                nc.sync.dma_start(out=xlt[64:128, 0:330], in_=xlt[0:64, 22:352])
                alq = blkp.tile([68, WBLK], BF16, tag="alq")
                nc.vector.tensor_copy(out=alq[64:68, :], in_=mmL[64:68, :])

                # --- pairwise scores, 128-partition (2 graphs per row-half) ---
                p2 = blkp.tile([128, 3872], BF16, tag="p2")
                in0 = xla[:].rearrange("p (k s) -> p k s", s=44)[:, :, 0:NPG] \
                    .unsqueeze(3).broadcast_to([128, 8, NPG, NPG])
                in1 = xra[:].rearrange("p (k s) -> p k s", s=44)[:, :, 0:NPG] \
                    .unsqueeze(2).broadcast_to([128, 8, NPG, NPG])
                nc.vector.tensor_tensor(out=p2[:], in0=in0, in1=in1, op=OP.add)
                nc.scalar.activation(out=p2[:], in_=p2[:], func=AF.Abs)

                ea = blkp.tile([128, 3872], F32, tag="ea")
                for grp in range(2):
                    sps = psB.tile([128, 4, 512], F32, tag="sps")
                    for gg in range(8):
                        k = grp * 4 + gg // 2
                        g2 = gg % 2
                        h0 = g2 * 64
                        out_ap = sps[h0:h0 + 64, gg // 2, 0:E1]
                        nc.tensor.matmul(out_ap, sgn_t[l][h0:h0 + 64, :],
                                         p2[h0:h0 + 64, k * E1:(k + 1) * E1],
                                         start=True, stop=False)
                    for gg in range(8):
                        k = grp * 4 + gg // 2
                        g2 = gg % 2
                        h0 = g2 * 64
                        out_ap = sps[h0:h0 + 64, gg // 2, 0:E1]
                        cg = k * 44 + g2 * NPG
                        alv = alq[64:68, cg:cg + NPG].unsqueeze(2) \
                            .broadcast_to([4, NPG, NPG])
                        nc.tensor.matmul(out_ap, alw_t[l][64:68, :], alv,
                                         start=False, stop=True)
                    nc.scalar.activation(
                        out=ea[:, grp * 1936:(grp + 1) * 1936]
                            .rearrange("p (k e) -> p k e", e=E1),
                        in_=sps[:, :, 0:E1], func=AF.Exp, scale=0.4)
                # ea cols: (kk: 8, e1: 484), graph = blk*16 + 2*kk + g2(row half)

                den = blkp.tile([128, 176], F32, tag="den")
                nc.vector.tensor_reduce(
                    out=den[:],
                    in_=ea[:].rearrange("p (k i j) -> p k j i", i=NPG, j=NPG),
                    axis=AX.X, op=OP.add)
                rden = blkp.tile([128, 176], F32, tag="rden")
                nc.vector.reciprocal(out=rden[:], in_=den[:])

                if last:
                    alp = zp.tile([128, 3872], F32, tag="alp")
                    rdv = rden[:].rearrange("p (k j) -> p k j", j=NPG) \
                        .unsqueeze(2).broadcast_to([128, 8, NPG, NPG])
                    nc.vector.tensor_tensor(out=alp[:], in0=ea[:], in1=rdv, op=OP.mult)
                    for g2 in range(2):
                        for h in range(4):
                            r = g2 * 64 + h * 16
                            in_a = alp[r:r + 1, :].rearrange("p (k e) -> p k e", e=E1)
                            out_a = bass.AP(alphat_d,
                                            h * EC + (blk * 16 + g2) * E1,
                                            [[2 * E1, 8], [1, E1]])
                            nc.sync.dma_start(out=out_a, in_=in_a)

                z = zp.tile([128, 3872], F32, tag="z")
                xltv = xlt[:].rearrange("p (k s) -> p k s", s=44)[:, :, 0:NPG] \
                    .unsqueeze(3).broadcast_to([128, 8, NPG, NPG])
                nc.gpsimd.tensor_tensor(out=z[:], in0=xltv, in1=ea[:], op=OP.mult)
                wsl = w_stack[:, blk * 176:(blk + 1) * 176]
                nc.vector.tensor_reduce(
                    out=wsl,
                    in_=z[:].rearrange("p (k i j) -> p k j i", i=NPG, j=NPG),
                    axis=AX.X, op=OP.add)
                nc.vector.tensor_tensor(out=wsl, in0=wsl, in1=rden[:], op=OP.mult)
                if last:
                    nc.vector.tensor_scalar_add(wsl, wsl, bias_t[l][:])
                else:
                    nc.vector.tensor_scalar(out=wsl, in0=wsl, scalar1=bias_t[l][:],
                                            scalar2=0.0, op0=OP.add, op1=OP.max)

            psB.release()
            psA.release()
            zp.release()
            blkp.release()

            # --- GraphNorm over w_stack [128, (blk,kk,j)] ---
            gnp = tc.alloc_tile_pool(name=f"gn{l}", bufs=1)
            NCG = NBLK * 8  # 256 graph-cols (pairs of graphs per row-half)
            wv = w_stack[:].rearrange("p (G j) -> p G j", j=NPG)
            mean = gnp.tile([128, NCG], F32)
            nc.vector.tensor_reduce(out=mean[:], in_=wv, axis=AX.X, op=OP.add)
            mmean = gnp.tile([128, NCG], F32)
            nc.vector.tensor_scalar_mul(mmean[:], mean[:], gnm_t[l][:])
            nc.vector.tensor_scalar_mul(mmean[:], mmean[:], 1.0 / NPG)
            cent = gnp.tile([128, NBLK * 176], F32)
            nc.vector.tensor_tensor(
                out=cent[:].rearrange("p (G j) -> p G j", j=NPG), in0=wv,
                in1=mmean[:].unsqueeze(2).broadcast_to([128, NCG, NPG]),
                op=OP.subtract)
            sq = gnp.tile([128, NBLK * 176], F32)
            nc.vector.tensor_tensor(out=sq[:], in0=cent[:], in1=cent[:], op=OP.mult)
            var = gnp.tile([128, NCG], F32)
            nc.vector.tensor_reduce(
                out=var[:], in_=sq[:].rearrange("p (G j) -> p G j", j=NPG),
                axis=AX.X, op=OP.add)
            nc.vector.tensor_scalar(out=var[:], in0=var[:], scalar1=1.0 / NPG,
                                    scalar2=EPS_GN, op0=OP.mult, op1=OP.add)
            sd = gnp.tile([128, NCG], F32)
            nc.scalar.activation(out=sd[:], in_=var[:], func=AF.Sqrt)
            inv = gnp.tile([128, NCG], F32)
            nc.vector.reciprocal(out=inv[:], in_=sd[:])
            winv = gnp.tile([128, NCG], F32)
            nc.vector.tensor_scalar_mul(winv[:], inv[:], gnw_t[l][:])
            gout = gnp.tile([128, NBLK * 176], F32)
            nc.vector.tensor_tensor(
                out=gout[:].rearrange("p (G j) -> p G j", j=NPG),
                in0=cent[:].rearrange("p (G j) -> p G j", j=NPG),
                in1=winv[:].unsqueeze(2).broadcast_to([128, NCG, NPG]), op=OP.mult)
            nc.vector.tensor_scalar_add(gout[:], gout[:], gnb_t[l][:])

            if not last:
                # XT-next: node col = (2*G + g2)*22 + j = G*44 + g2*22 + j
                for g2 in range(2):
                    in_g = gout[g2 * 64:g2 * 64 + 64, :] \
                        .rearrange("p (G j) -> p G j", j=NPG)
                    out_g = bass.AP(dst.tensor, dst.offset + g2 * NPG,
                                    [[NNODE, 64], [44, NCG], [1, NPG]])
                    nc.sync.dma_start(out=out_g, in_=in_g)
            else:
                pooled_t = gnp.tile([128, NCG], F32)
                nc.vector.tensor_reduce(
                    out=pooled_t[:], in_=gout[:].rearrange("p (G j) -> p G j", j=NPG),
                    axis=AX.X, op=OP.add)
                nc.vector.tensor_scalar_mul(pooled_t[:], pooled_t[:], 1.0 / NPG)
                nc.sync.dma_start(out=pooledt_d[:], in_=pooled_t[:])
                with tc.tile_pool(name="ops", bufs=1, space="PSUM") as opsps:
                    osb = gnp.tile([2, 512], F32)
                    for g2 in range(2):
                        ops = opsps.tile([2, NCG], F32, tag=f"o{g2}")
                        nc.tensor.matmul(ops[:], linw_t[g2 * 64:g2 * 64 + 64, :],
                                         pooled_t[g2 * 64:g2 * 64 + 64, :],
                                         start=True, stop=True)
                        nc.vector.tensor_scalar_add(
                            osb[:, g2 * NCG:(g2 + 1) * NCG], ops[:], linb_t[:])
                    nc.sync.dma_start(out=ot_d[:], in_=osb[:])
            gnp.release()
            lp.release()

        consts.release()
        dram.release()

    nc.finalize()
    return nc


def _preprocess(inputs):
    import ml_dtypes
    x = np.ascontiguousarray(np.asarray(inputs["x"], np.float32))
    common = {}
    for li, l in enumerate(("1", "2", "3")):
        Wl = np.asarray(inputs[f"Wl{l}"], np.float32)
        bl = np.asarray(inputs[f"bl{l}"], np.float32)
        Wr = np.asarray(inputs[f"Wr{l}"], np.float32)
        br = np.asarray(inputs[f"br{l}"], np.float32)
        att = np.asarray(inputs[f"att{l}"], np.float32)   # [H, C]
        WlA = np.einsum("dhc,hc->dh", Wl.reshape(-1, H, C), att)
        blA = np.einsum("hc,hc->h", bl.reshape(H, C), att)
        common[f"wl{li}"] = np.concatenate(
            [np.vstack([Wl, bl[None, :]]),
             np.vstack([WlA, blA[None, :]])], axis=1).astype(np.float32)
        common[f"wr{li}"] = np.vstack([Wr, br[None, :]]).astype(np.float32)
        sgn = np.zeros((64, 64), np.float32)
        s = np.sign(att)
        for h in range(H):
            sgn[h * C:(h + 1) * C, h * C:(h + 1) * C] = s[h][:, None]
        common[f"sgn{li}"] = np.tile(sgn, (2, 1)).astype(ml_dtypes.bfloat16)
        alw = np.zeros((68, 64), np.float32)
        for h in range(H):
            alw[64 + h, h * C:(h + 1) * C] = 1.5
        common[f"alw{li}"] = alw.astype(ml_dtypes.bfloat16)
        common[f"aa{li}"] = np.abs(att).reshape(64, 1).astype(np.float32)
        common[f"bias{li}"] = np.tile(np.asarray(inputs[f"bias{l}"], np.float32), 2)[:, None].copy()
        common[f"gnw{li}"] = np.tile(np.asarray(inputs[f"gnw{l}"], np.float32), 2)[:, None].copy()
        common[f"gnb{li}"] = np.tile(np.asarray(inputs[f"gnb{l}"], np.float32), 2)[:, None].copy()
        common[f"gnm{li}"] = np.tile(np.asarray(inputs[f"gnm{l}"], np.float32), 2)[:, None].copy()
    common["linw"] = np.tile(np.asarray(inputs["linW"], np.float32), (2, 1)).copy()
    common["linb"] = np.asarray(inputs["linb"], np.float32).reshape(2, 1).copy()
    common["ident"] = np.eye(128, dtype=np.float32)
    in_maps = []
    for c in range(N_CORES):
        m = dict(common)
        m["x"] = x[c * NNODE:(c + 1) * NNODE].copy()
        in_maps.append(m)
    return in_maps


def kernel(**inputs):
    from concourse.bass_utils import run_bass_kernel_spmd

    if "nc" not in _PROG:
        _PROG["nc"] = _build_program()
    nc = _PROG["nc"]
    in_maps = _preprocess(inputs)
    res = run_bass_kernel_spmd(nc, in_maps, list(range(N_CORES))).results

    o = np.empty((4096, 2), np.float32)
    pooled = np.empty((4096, F1), np.float32)
    alpha3 = np.empty((4096 * E1, H), np.float32)
    bg = np.arange(256)
    for c in range(N_CORES):
        r = res[c]
        alpha3[c * EC:(c + 1) * EC] = r["alphat"].T
        pt = r["pooledt"]
        ot = r["ot"]
        for g2 in range(2):
            g = c * NG + 2 * bg + g2
            pooled[g] = pt[g2 * 64:(g2 + 1) * 64, :][:, bg].T
            o[g] = ot[:, g2 * 256 + bg].T
    return (o, pooled, alpha3)
